# revision 4
# baseline (speedup 1.0000x reference)
"""Trainium2 Bass kernel: GQA sliding-window attention (v3, fp16).

Problem: B=1, T=4096, D=2048, H=16 q-heads, KVH=4 kv-heads, HD=128,
causal sliding window 512.

Sharding: 8-way sequence parallel. Core c owns query rows
[512c, 512c+512). It receives x rows [512(c-1), 512(c+1)) (halo of 512
rows; core 0's halo is zeros). Weights replicated. Outputs are disjoint
row blocks -> plain concatenation, no collectives.

v3 changes vs v2: the PE transpose stage is gone. Scores are computed
TRANSPOSED (sT[j,i] = k_j.q_i via stationary kT chunks streaming qT
columns -- same PE cost as forward scores since both q/k live in
[hd-partition, rows] layout), so exp writes wT straight to SBUF in the
exact layout PV consumes. What the forward orientation used to give for
free (row sums via exp accum_out) is recovered with per-tile N=1
matmuls (out [128,1] = wT_chunk.T @ ones): 20 one-column matmuls per
head whose stationary loads the cost model does not charge. The 1/l
normalization moves to the PV output: r = 1/(l+lcorr) is PE-transposed
[128,4]->[4,128], partition-broadcast on Pool to [128,128] tiles, and
applied as one DVE multiply on the PV PSUM during the oT copy-out.

Per-head PE work drops from 7680 cols (scores 2560 + transposes 2560 +
PV 2560) to ~5250 (scoresT 2560 + PV 2560 + rT 128 + l ~0): -38k
charged columns overall. DVE drops the 59us of pt->wT copies; ACT
loses accum_out but keeps the same exp element count.

Transposed-score mask: band block (jc, t) with co = jc - t needs no
mask for co in 1..3; co=0 needs allowed jj>=ii (same triangle as v2's
maskL), co=4 needs jj<=ii (v2's maskR). Chunks are paired (jc, jc+4)
in one [128,1024] PSUM tile (block counts 1+4, 2+3, 3+2, 4+1 -- always
640 cols), with both edge blocks hit by ONE strided DVE add of the
same stacked [128,2,128] mask tile as v2.

Per-core layouts (SBUF partition dim first, all fp16 except f32 sums):
  xt  [128, 16, 1024] : xt[p, dc, j] = x_c[j, 128*dc+p]   (host prep)
  qT  [128, 16, 512]  : qT[p, h, i]  = q[i, 128*h+p]  (unscaled)
  kT  [128, 4, 1024]  : kT[p, g, j]  = k[j, 128*g+p]
  vv  [128, 8, 512]   : vv[p, jc, e] = v[128*jc+p, e]
  wT  [128, 8, 512]   : wT[p, jc, i] = exp(s*SCALE)[i, 128*jc+p]
  oT  [128, 16, 512]  : oT[p, h, i]  = attn_out[i, 128*h+p]
  y = oT.T @ Wo accumulated over heads, streamed in 512-col blocks,
  written fp16 and upcast host-side.
"""

import numpy as np

T = 4096
D = 2048
H = 16
KVH = 4
HD = 128
WINDOW = 512
SCALE = HD ** -0.5
N_CORES = 8
TLOC = T // N_CORES          # 512 own query rows / core
XROWS = TLOC + WINDOW        # 1024 x rows / core (halo + own)
NT = TLOC // 128             # 4 q-tiles of 128 rows
NJC = XROWS // 128           # 8 key chunks of 128
BAND = WINDOW + 128          # 640 key columns per q-tile
DC = D // 128                # 16 d-chunks
N_REP = H // KVH
MASK_VAL = -1e9

_CACHE = {}


def _emit(nc, tc, tile, mybir, make_identity, loop_n=None, stop_after=None):
    f32 = mybir.dt.float32
    f16 = mybir.dt.float16

    timing = loop_n is not None
    kin = "Internal" if timing else "ExternalInput"
    kout = "Internal" if timing else "ExternalOutput"
    xt_d = nc.dram_tensor("xt", [128, DC * XROWS], f16, kind=kin)
    wq_d = nc.dram_tensor("wq", [128, H * DC * 128], f16, kind=kin)
    wk_d = nc.dram_tensor("wk", [128, KVH * DC * 128], f16, kind=kin)
    wv_d = nc.dram_tensor("wv", [128, DC * 512], f16, kind=kin)
    wo_d = nc.dram_tensor("wo", [128, 4 * H * 512], f16, kind=kin)
    lcorr_d = nc.dram_tensor("lcorr", [128, NT], f32, kind=kin)
    y_d = nc.dram_tensor("y", [TLOC, D], f16, kind=kout)
    if timing:
        dummy_d = nc.dram_tensor("bench_done", [1, 128], f32,
                                 kind="ExternalOutput")

    def mm(out, lhsT, rhs, start, stop):
        nc.tensor.matmul(out, lhsT, rhs, start=start, stop=stop)

    # --- long-lived pools / loop-invariant tiles ---
    # PSUM budget (8 banks): ps_s 2 bufs x [128,640->1024]f32 = 4 banks,
    # ps_ot 2 bufs x [128,512]f32 = 2 banks, plus one phase-scoped
    # right-side pool of <=2 banks (ps_acc in P1, ps_l [l + rT] in P2,
    # ps_acc2 in P3).
    pers = tc.alloc_tile_pool(name="pers", bufs=1)
    ps_s = tc.alloc_tile_pool(name="ps_s", bufs=2, space="PSUM")
    ps_ot = tc.alloc_tile_pool(name="ps_ot", bufs=2, space="PSUM")

    ident32 = pers.tile([128, 128], f32, tag="ident32")
    make_identity(nc, ident32[:])
    ones = pers.tile([128, 1], f16, tag="ones")
    nc.gpsimd.memset(ones[:], 1.0)
    # additive edge masks, stacked [128, 2, 128], TRANSPOSED orientation
    # (partition p = key jj, free col = query ii): block 0 = co=0 edge
    # (allowed jj >= ii, keep p >= col), block 1 = co=4 edge (allowed
    # jj <= ii, keep col >= p). Applied as ONE strided DVE add per
    # chunk pair.
    masks = pers.tile([128, 2, 128], f32, tag="masks")
    nc.gpsimd.memset(masks[:], 0.0)
    nc.gpsimd.affine_select(
        out=masks[:, 0, :], in_=masks[:, 0, :],
        compare_op=mybir.AluOpType.is_ge,
        fill=MASK_VAL, base=0, pattern=[[-1, 128]], channel_multiplier=1)
    nc.gpsimd.affine_select(
        out=masks[:, 1, :], in_=masks[:, 1, :],
        compare_op=mybir.AluOpType.is_ge,
        fill=MASK_VAL, base=0, pattern=[[1, 128]], channel_multiplier=-1)

    lp = tc.For_i(0, loop_n, 1) if timing else None
    if lp is not None:
        lp.__enter__()

    proj = tc.alloc_tile_pool(name="proj", bufs=1)
    xp = tc.alloc_tile_pool(name="xp", bufs=1)
    wp = tc.alloc_tile_pool(name="wpool", bufs=2)
    ps_acc = tc.alloc_tile_pool(name="ps_acc", bufs=2, space="PSUM",
                                side="right")

    qT = proj.tile([128, H, TLOC], f16, tag="qT")
    kT = proj.tile([128, KVH, XROWS], f16, tag="kT")
    vv = proj.tile([128, NJC, KVH * HD], f16, tag="vv")
    lcorr_s = proj.tile([128, NT], f32, tag="lcorr")
    xt = xp.tile([128, DC, XROWS], f16, tag="xt")

    nc.sync.dma_start(lcorr_s[:], lcorr_d.ap())

    # ---------------- P1a: k projections (x streamed in) -------------
    # The prologue is DMA-bound (serial transfer resource), so k-proj
    # runs dc-OUTER with all 8 (kv-head, half) accumulation groups open
    # at once across all 8 PSUM banks: each arriving x chunk is fully
    # consumed (8 x 512-col matmuls) before the next chunk lands.
    # DMA order = consumption order: wk g0/g1, x chunks, wk g2/g3, wv.
    wkgs = []
    for g in range(KVH):
        wkg = wp.tile([128, DC, 128], f16, tag="wlhs", name=f"wkg{g}",
                      bufs=4)
        wkgs.append(wkg)
    # first dc-slice of wk g0 split out so the very first matmul can
    # start ~1.5us earlier on the serial DMA stream
    nc.sync.dma_start(wkgs[0][:, 0, :], wk_d.ap()[:, 0:128])
    nc.sync.dma_start(xt[:, 0, :], xt_d.ap()[:, 0:XROWS])
    nc.sync.dma_start(wkgs[0][:, 1:DC, :], wk_d.ap()[:, 128:DC * 128])
    for dc in range(1, DC):
        nc.sync.dma_start(xt[:, dc, :],
                          xt_d.ap()[:, dc * XROWS:(dc + 1) * XROWS])
        if dc == 1:
            # g1 weights after the first x chunks: g0's matmuls cover
            # the PE meanwhile
            nc.sync.dma_start(wkgs[1][:],
                              wk_d.ap()[:, DC * 128:2 * DC * 128])
    nc.sync.dma_start(wkgs[2][:], wk_d.ap()[:, 2 * DC * 128:3 * DC * 128])
    nc.sync.dma_start(wkgs[3][:], wk_d.ap()[:, 3 * DC * 128:4 * DC * 128])

    def kproj_pair(g0, g1, slots, warmup=0):
        """dc-outer over two kv heads: 4 open accumulation groups;
        each x chunk fully consumed on arrival (4 x 512-col matmuls ~
        one chunk's DMA time). warmup: emit g0's first `warmup` chunks
        before g1's so the in-order PE queue isn't blocked on g1's
        weight DMA at startup."""
        gs = [g0, g0, g1, g1]
        for dc in range(warmup):
            for s in range(2):
                mm(slots[s], wkgs[gs[s]][:, dc, :],
                   xt[:, dc, (s % 2) * 512:(s % 2 + 1) * 512],
                   start=(dc == 0), stop=(dc == DC - 1))
        for dc in range(warmup):
            for s in range(2, 4):
                mm(slots[s], wkgs[gs[s]][:, dc, :],
                   xt[:, dc, (s % 2) * 512:(s % 2 + 1) * 512],
                   start=(dc == 0), stop=(dc == DC - 1))
        for dc in range(warmup, DC):
            for s in range(4):
                mm(slots[s], wkgs[gs[s]][:, dc, :],
                   xt[:, dc, (s % 2) * 512:(s % 2 + 1) * 512],
                   start=(dc == 0), stop=(dc == DC - 1))
        for s in range(4):
            # GPSIMD cannot access PSUM on HW: copies go DVE/ACT only
            dst = kT[:, gs[s], (s % 2) * 512:(s % 2 + 1) * 512]
            if s % 2 == 0:
                nc.vector.tensor_copy(dst, slots[s])
            else:
                nc.scalar.copy(dst, slots[s])

    # phase A in 2x[128,1024] ps_s tiles; phase B in ps_ot/ps_acc slots
    # so it does not wait on phase A's PSUM->SBUF copies.
    pkA = [ps_s.tile([128, 1024], f32, tag="score", name=f"pkA{i}")
           for i in range(2)]
    kproj_pair(0, 1, [pkA[0][:, 0:512], pkA[0][:, 512:1024],
                      pkA[1][:, 0:512], pkA[1][:, 512:1024]], warmup=3)
    pkO = [ps_ot.tile([128, TLOC], f32, tag="ot", name=f"pkO{i}")
           for i in range(2)]
    pkB = [ps_acc.tile([128, 512], f32, tag="acc", name=f"pkB{i}")
           for i in range(2)]
    kproj_pair(2, 3, [pkO[0][:], pkO[1][:], pkB[0][:], pkB[1][:]])

    # ---------------- P1b: v projections ------------------------------
    # chunk 7 is deferred into head-0's iteration as PE filler (the
    # pipelined head loop has no PV/rT work for h=0 yet).
    wvt = wp.tile([128, DC, 512], f16, tag="wv", name="wvt", bufs=1)
    nc.sync.dma_start(wvt[:], wv_d.ap())
    for jc in range(NJC - 1):
        pv = ps_acc.tile([128, 512], f32, tag="acc")
        for dc in range(DC):
            mm(pv[:], xt[:, dc, jc * 128:(jc + 1) * 128], wvt[:, dc, :],
               start=(dc == 0), stop=(dc == DC - 1))
        nc.vector.tensor_copy(vv[:, jc, :], pv[:])

    ps_acc.release()

    if stop_after == "kv":
        if lp is not None:
            lp.__exit__(None, None, None)
            dtile = pers.tile([128, 128], f32, tag="dtile")
            nc.vector.memset(dtile[:], 0.0)
            nc.sync.dma_start(dummy_d.ap(), dtile[0:1, :])
        wp.release()
        xp.release()
        proj.release()
        ps_ot.release()
        ps_s.release()
        pers.release()
        return

    # ---------------- P2: attention, q projection interleaved ---------
    attn = tc.alloc_tile_pool(name="attn", bufs=1, side="right")
    # Wo stream pool allocated before sm/ps_l (right-side pools are
    # released in stack order) so the first two 2 MiB chunks can be
    # prefetched during the attention tail.
    wop = tc.alloc_tile_pool(name="wo_pool", bufs=2, side="right")
    sm = tc.alloc_tile_pool(name="sm", bufs=2, side="right")
    ps_l = tc.alloc_tile_pool(name="ps_l", bufs=1, space="PSUM",
                              side="right")

    oT = attn.tile([128, H, TLOC], f16, tag="oT")
    # wT double-buffered by head parity: exps of head h write wTs[h%2]
    # while PV of head h-1 still reads wTs[(h-1)%2].
    wTs = [attn.tile([128, NJC, TLOC], f16, tag=f"wT{i}", name=f"wT{i}")
           for i in range(2)]

    woc_tiles = {}

    def wo_load(dblk):
        woc = wop.tile([128, H, 512], f16, tag="wo", name=f"wo{dblk}")
        nc.sync.dma_start(
            woc[:], wo_d.ap()[:, dblk * H * 512:(dblk + 1) * H * 512])
        woc_tiles[dblk] = woc

    # --- per-head emission pieces (closures over head state) ---------

    def make_scores_T(h):
        """Transposed-score emitters for head h. pair(pi) computes the
        sT blocks of chunks (pi, pi+4) in one [128,1024] PSUM tile
        (2 matmuls streaming qT columns), applies both edge masks with
        one strided DVE add, and exps into wT[:, jc, .] directly.
        lsum(t) accumulates the masked row sums via 5 N=1 matmuls.
        fin() adds lcorr and takes the reciprocal."""
        g = h // N_REP
        wT = wTs[h % 2]
        l_ps = ps_l.tile([128, NT], f32, tag="l", name=f"l{h}", bufs=1)

        def pair(pi):
            jc0, jc1 = pi, pi + 4
            n0, n1 = pi + 1, 4 - pi
            ps = ps_s.tile([128, 1024], f32, tag="score",
                           name=f"sT{h}_{pi}")
            # chunk jc0 covers q-tiles 0..pi at cols [0, n0*128);
            # chunk jc1 covers q-tiles pi..3 at cols [512, 512+n1*128)
            mm(ps[:, 0:n0 * 128], kT[:, g, jc0 * 128:(jc0 + 1) * 128],
               qT[:, h, 0:n0 * 128], start=True, stop=True)
            mm(ps[:, 512:512 + n1 * 128],
               kT[:, g, jc1 * 128:(jc1 + 1) * 128],
               qT[:, h, pi * 128:TLOC], start=True, stop=True)
            # both edge blocks (at cols pi*128 and 512) in one strided
            # DVE add of the stacked masks tile
            s = 512 - pi * 128
            edges = ps[:, pi * 128:pi * 128 + 2 * s].rearrange(
                "p (a b) -> p a b", a=2)[:, :, 0:128]
            nc.vector.tensor_add(edges, edges, masks[:])
            nc.scalar.activation(wT[:, jc0, 0:n0 * 128], ps[:, 0:n0 * 128],
                                 mybir.ActivationFunctionType.Exp,
                                 scale=SCALE)
            nc.scalar.activation(wT[:, jc1, pi * 128:TLOC],
                                 ps[:, 512:512 + n1 * 128],
                                 mybir.ActivationFunctionType.Exp,
                                 scale=SCALE)

        def lsum(t):
            # row sums l[i] for q-tile t: 5 one-column matmuls
            # (stationary = wT chunk, moving = ones) accumulating into
            # l_ps[:, t]. Stationary loads are pipelined; the charged
            # stream is 1 column per matmul.
            for i, jc in enumerate(range(t, t + 5)):
                mm(l_ps[:, t:t + 1], wT[:, jc, t * 128:(t + 1) * 128],
                   ones[:], start=(i == 0), stop=(i == 4))

        def fin():
            lf = sm.tile([128, NT], f32, tag="lf", name=f"lf{h}", bufs=2)
            r = sm.tile([128, NT], f32, tag="r", name=f"r{h}", bufs=2)
            nc.vector.tensor_add(lf[:], l_ps[:], lcorr_s[:])
            nc.vector.reciprocal(r[:], lf[:])
            return r

        return pair, lsum, fin, (h, wT)

    def emit_rchain(h, r):
        """r [128, NT] -> flat [1, NT*128] row on partition 0 (4 PE
        column transposes) -> SBUF -> [128, NT*128] partition-broadcast
        tiles (Pool) for the PV-output multiply. The flat layout keeps
        every broadcast input at partition 0 (a BIR requirement)."""
        rT_ps = ps_l.tile([1, NT * 128], f32, tag="rT", name=f"rT{h}",
                          bufs=1)
        for t in range(NT):
            nc.tensor.transpose(rT_ps[0:1, t * 128:(t + 1) * 128],
                                r[:, t:t + 1], ident32[:])
        rTs = sm.tile([1, NT * 128], f32, tag="rTs", name=f"rTs{h}",
                      bufs=2)
        nc.vector.tensor_copy(rTs[:], rT_ps[:])
        rbc = sm.tile([128, NT, 128], f32, tag="rbc", name=f"rbc{h}",
                      bufs=2)
        for t in range(NT):
            nc.gpsimd.partition_broadcast(
                rbc[:, t, :], rTs[0:1, t * 128:(t + 1) * 128])
        return rbc

    def make_qp(hq):
        """q-projection matmul chunks for head hq (PE filler)."""
        if hq >= H:
            return (lambda lo, hi: None), (lambda: None)
        wqh = wp.tile([128, DC, 128], f16, tag="wlhs", name=f"wqh{hq}",
                      bufs=4)
        nc.sync.dma_start(
            wqh[:], wq_d.ap()[:, hq * DC * 128:(hq + 1) * DC * 128])
        pq = ps_ot.tile([128, TLOC], f32, tag="ot", name=f"pq{hq}")

        def qp_mms(dc_lo, dc_hi):
            for dc in range(dc_lo, dc_hi):
                mm(pq[:], wqh[:, dc, :], xt[:, dc, WINDOW:XROWS],
                   start=(dc == 0), stop=(dc == DC - 1))

        def qp_fin():
            # (GPSIMD cannot access PSUM on HW)
            nc.scalar.copy(qT[:, hq, :], pq[:])

        return qp_mms, qp_fin

    def make_pv(state):
        """banded PV emitters for head h, reading wTs[h%2] (whose exps
        completed last iteration) + the normalizing oT multiply."""
        h, wT = state
        g = h // N_REP
        po = ps_ot.tile([128, TLOC], f32, tag="ot", name=f"po{h}")

        def pv(t):
            # one accumulation group open at a time per PSUM bank
            for i, jc in enumerate(range(t, t + 5)):
                mm(po[:, t * 128:(t + 1) * 128],
                   vv[:, jc, g * 128:(g + 1) * 128],
                   wT[:, jc, t * 128:(t + 1) * 128],
                   start=(i == 0), stop=(i == 4))

        def ot_fin(rbc):
            # oT[:, h, :] = po * r_i in ONE DVE multiply (the PSUM read
            # that used to be a plain copy now also normalizes)
            nc.vector.tensor_mul(
                oT[:, h, :], po[:],
                rbc[:].rearrange("p a b -> p (a b)"))

        return pv, ot_fin

    py_tiles = {}  # t -> (tile, col half)

    def py_open(dblk):
        pyA = ps_s.tile([128, 1024], f32, tag="score", name=f"pyA{dblk}")
        pyB = ps_s.tile([128, 1024], f32, tag="score", name=f"pyB{dblk}")
        for t in range(NT):
            py_tiles[t] = (pyA if t < 2 else pyB, t % 2)

    def py_mms(t, h_lo, h_hi, dblk):
        woc = woc_tiles[dblk]
        py, half = py_tiles[t]
        for h2 in range(h_lo, h_hi):
            mm(py[:, half * 512:(half + 1) * 512],
               oT[:, h2, t * 128:(t + 1) * 128], woc[:, h2, :],
               start=(h2 == 0), stop=(h2 == H - 1))

    skip_p3 = stop_after == "attn"

    def PY(t, h_lo, h_hi):
        if not skip_p3:
            py_mms(t, h_lo, h_hi, 0)

    def vv_late(jc, eng):
        """deferred v-projection chunk (PE filler in iteration 0)"""
        pvv = ps_ot.tile([128, TLOC], f32, tag="ot", name=f"pvv{jc}")
        for dc in range(DC):
            mm(pvv[:], xt[:, dc, jc * 128:(jc + 1) * 128],
               wvt[:, dc, :], start=(dc == 0), stop=(dc == DC - 1))
        if eng == 0:
            nc.vector.tensor_copy(vv[:, jc, :], pvv[:])
        else:
            nc.scalar.copy(vv[:, jc, :], pvv[:])

    # --- pipelined head loop -----------------------------------------
    # Iteration h emits: scoresT+exp of head h, q-proj of h+1, PV and
    # normalized oT copy-out of h-1, r-chain of h-1 (transpose early so
    # the Pool broadcasts finish before ot_fin needs them).

    # head 0's q-projection runs standalone (heads h+1 ride iteration h)
    qp0_mms, qp0_fin = make_qp(0)
    qp0_mms(0, DC)
    qp0_fin()

    prev = None   # (h, wT) and r for head h-1 (PV + normalize stage)
    for h in range(H):
        pair, lsum, fin, cur_state = make_scores_T(h)
        qp_mms, qp_fin = make_qp(h + 1)
        if prev is not None:
            pstate, pr = prev
            pv, ot_fin = make_pv(pstate)
        else:
            pv = ot_fin = None

        def PV(t):
            if pv is not None:
                pv(t)

        pair(0)
        PV(0)
        if prev is not None:
            rbc = emit_rchain(pstate[0], pr)
        pair(1)
        PV(1)
        pair(2)
        PV(2)
        pair(3)
        PV(3)
        if h == 0:
            vv_late(7, 1)
        if h == H - 1 and not skip_p3:
            # no q-projection filler for a 17th head: use Wo block 0's
            # first partial accumulations instead (pyA/pyB bind to the
            # score buffers freed by this head's own exps)
            py_open(0)
            PY(0, 0, 7)
            qp_mms(0, DC)  # no-op (h+1 == H)
        else:
            qp_mms(0, 8)
        if ot_fin is not None:
            ot_fin(rbc)
        qp_mms(8, DC)
        for t in range(NT):
            lsum(t)
        r_cur = fin()
        qp_fin()
        if h == 12:
            wo_load(0)
        elif h == 14:
            wo_load(1)
        prev = (cur_state, r_cur)

    # --- drain: PV + normalize of head 15, with the first Wo block's
    # partial accumulations (heads 0..13) as PE filler.
    pstate, pr = prev
    pv15, ot_fin15 = make_pv(pstate)
    pv15(0)
    rbc15 = emit_rchain(pstate[0], pr)
    PY(0, 7, 14)
    pv15(1)
    PY(1, 0, 7)
    pv15(2)
    PY(1, 7, 14)
    pv15(3)
    PY(2, 0, 7)
    ot_fin15(rbc15)
    PY(2, 7, 14)
    PY(3, 0, 7)
    PY(3, 7, 14)

    sm.release()
    ps_l.release()
    wp.release()
    xp.release()
    proj.release()

    if stop_after == "attn":
        if lp is not None:
            lp.__exit__(None, None, None)
            dtile = pers.tile([128, 128], f32, tag="dtile")
            nc.vector.memset(dtile[:], 0.0)
            nc.sync.dma_start(dummy_d.ap(), dtile[0:1, :])
        wop.release()
        attn.release()
        ps_ot.release()
        ps_s.release()
        pers.release()
        return

    # ---------------- P3: output projection ----------------
    # dblk 0's heads 0..13 already accumulated during the drain above;
    # finish with heads 14/15, then stream the remaining Wo blocks.
    def y_out(t, dblk, py_ap):
        ych = attn.tile([128, 512], f16, tag="ych", bufs=3)
        nc.vector.tensor_copy(ych[:], py_ap)
        nc.scalar.dma_start(
            y_d.ap()[t * 128:(t + 1) * 128,
                     dblk * 512:(dblk + 1) * 512],
            ych[:])

    # all h=14 contributions first (oT14 ready early), so the PE has
    # work while the normalized oT15 multiply drains
    for t in range(NT):
        py_mms(t, H - 2, H - 1, 0)
    for t in range(NT):
        py_mms(t, H - 1, H, 0)
        py, half = py_tiles[t]
        y_out(t, 0, py[:, half * 512:(half + 1) * 512])

    for dblk in range(1, 4):
        if dblk not in woc_tiles:
            wo_load(dblk)
        woc = woc_tiles[dblk]
        for t in range(NT):
            # alternate PSUM pools between dblks so a block's first
            # matmuls never wait on the previous block's output copies
            if dblk % 2 == 0:
                py = ps_s.tile([128, 1024], f32, tag="score",
                               name=f"py{dblk}_{t}")
                py_ap = py[:, 0:512]
            else:
                py = ps_ot.tile([128, TLOC], f32, tag="ot",
                                name=f"py{dblk}_{t}")
                py_ap = py[:]
            for h in range(H):
                mm(py_ap, oT[:, h, t * 128:(t + 1) * 128],
                   woc[:, h, :], start=(h == 0), stop=(h == H - 1))
            y_out(t, dblk, py_ap)

    wop.release()
    attn.release()

    if lp is not None:
        lp.__exit__(None, None, None)
        dtile = pers.tile([128, 128], f32, tag="dtile")
        nc.vector.memset(dtile[:], 0.0)
        nc.sync.dma_start(dummy_d.ap(), dtile[0:1, :])

    ps_ot.release()
    ps_s.release()
    pers.release()


def build_nc(loop_n=None, stop_after=None):
    key = ("nc", loop_n, stop_after)
    if key in _CACHE:
        return _CACHE[key]
    import concourse.bacc as bacc
    import concourse.mybir as mybir
    import concourse.tile as tile
    from concourse.masks import make_identity

    nc = bacc.Bacc("TRN2", target_bir_lowering=False, debug=False,
                   num_devices=N_CORES)
    with tile.TileContext(nc) as tc:
        _emit(nc, tc, tile, mybir, make_identity, loop_n=loop_n,
              stop_after=stop_after)
    nc.compile()
    _CACHE[key] = nc
    return nc


def make_inputs_for_core(c, xf, Wq, Wk, Wv, Wo):
    """xf: [T, D] float32 (already squeezed)."""
    f16 = np.float16
    if c == 0:
        x_c = np.concatenate(
            [np.zeros((WINDOW, D), np.float32), xf[:TLOC]], axis=0)
    else:
        x_c = xf[TLOC * c - WINDOW: TLOC * c + TLOC]

    # xt[p, dc, j] = x_c[j, 128*dc+p]
    xt = np.ascontiguousarray(
        x_c.reshape(XROWS, DC, 128).transpose(2, 1, 0).astype(f16)
    ).reshape(128, DC * XROWS)
    # wq[p, h, dc, e] = Wq[128*dc+p, 128*h+e]
    wq = np.ascontiguousarray(
        Wq.reshape(DC, 128, H, 128).transpose(1, 2, 0, 3).astype(f16)
    ).reshape(128, H * DC * 128)
    # wk[p, g, dc, e] = Wk[128*dc+p, 128*g+e]
    wk = np.ascontiguousarray(
        Wk.reshape(DC, 128, KVH, 128).transpose(1, 2, 0, 3).astype(f16)
    ).reshape(128, KVH * DC * 128)
    # wv[p, dc, e] = Wv[128*dc+p, e]
    wv = np.ascontiguousarray(
        Wv.reshape(DC, 128, KVH * HD).transpose(1, 0, 2).astype(f16)
    ).reshape(128, DC * 512)
    # wo[p, dblk, h, e] = Wo[128*h+p, 512*dblk+e]
    wo = np.ascontiguousarray(
        Wo.reshape(H, 128, 4, 512).transpose(1, 2, 0, 3).astype(f16)
    ).reshape(128, 4 * H * 512)

    # core 0: rows see (512 - i) spurious zero-halo keys, each exp(0)=1
    lcorr = np.zeros((128, NT), np.float32)
    if c == 0:
        p = np.arange(128)[:, None]
        t = np.arange(NT)[None, :]
        lcorr = -np.maximum(0, (512 - 128 * t) - p).astype(np.float32)

    return {
        "xt": xt,
        "wq": wq,
        "wk": wk,
        "wv": wv,
        "wo": wo,
        "lcorr": np.ascontiguousarray(lcorr),
    }


def kernel(x, Wq, Wk, Wv, Wo):
    from concourse.bass_utils import run_bass_kernel_spmd

    nc = build_nc()
    xf = np.asarray(x, np.float32).reshape(T, D)
    Wq = np.asarray(Wq, np.float32)
    Wk = np.asarray(Wk, np.float32)
    Wv = np.asarray(Wv, np.float32)
    Wo = np.asarray(Wo, np.float32)
    in_maps = [make_inputs_for_core(c, xf, Wq, Wk, Wv, Wo)
               for c in range(N_CORES)]
    res = run_bass_kernel_spmd(nc, in_maps, core_ids=list(range(N_CORES)))
    y = np.concatenate(
        [res.results[c]["y"].astype(np.float32) for c in range(N_CORES)],
        axis=0)
    return y.reshape(1, T, D)


# revision 7
# speedup vs baseline: 1.0188x; 1.0188x over previous
"""Trainium2 Bass kernel: GQA sliding-window attention (v3, fp16).

Problem: B=1, T=4096, D=2048, H=16 q-heads, KVH=4 kv-heads, HD=128,
causal sliding window 512.

Sharding: 8-way sequence parallel. Core c owns query rows
[512c, 512c+512). It receives x rows [512(c-1), 512(c+1)) (halo of 512
rows; core 0's halo is zeros). Weights replicated. Outputs are disjoint
row blocks -> plain concatenation, no collectives.

v3 changes vs v2: the PE transpose stage is gone. Scores are computed
TRANSPOSED (sT[j,i] = k_j.q_i via stationary kT chunks streaming qT
columns -- same PE cost as forward scores since both q/k live in
[hd-partition, rows] layout), so exp writes wT straight to SBUF in the
exact layout PV consumes. What the forward orientation used to give for
free (row sums via exp accum_out) is recovered with per-tile N=1
matmuls (out [128,1] = wT_chunk.T @ ones): 20 one-column matmuls per
head whose stationary loads the cost model does not charge. The 1/l
normalization moves to the PV output: r = 1/(l+lcorr) is PE-transposed
[128,4]->[4,128], partition-broadcast on Pool to [128,128] tiles, and
applied as one DVE multiply on the PV PSUM during the oT copy-out.

Per-head PE work drops from 7680 cols (scores 2560 + transposes 2560 +
PV 2560) to ~5250 (scoresT 2560 + PV 2560 + rT 128 + l ~0): -38k
charged columns overall. DVE drops the 59us of pt->wT copies; ACT
loses accum_out but keeps the same exp element count.

Transposed-score mask: band block (jc, t) with co = jc - t needs no
mask for co in 1..3; co=0 needs allowed jj>=ii (same triangle as v2's
maskL), co=4 needs jj<=ii (v2's maskR). Chunks are paired (jc, jc+4)
in one [128,1024] PSUM tile (block counts 1+4, 2+3, 3+2, 4+1 -- always
640 cols), with both edge blocks hit by ONE strided DVE add of the
same stacked [128,2,128] mask tile as v2.

Per-core layouts (SBUF partition dim first, all fp16 except f32 sums):
  xt  [128, 16, 1024] : xt[p, dc, j] = x_c[j, 128*dc+p]   (host prep)
  qT  [128, 16, 512]  : qT[p, h, i]  = q[i, 128*h+p]  (unscaled)
  kT  [128, 4, 1024]  : kT[p, g, j]  = k[j, 128*g+p]
  vv  [128, 8, 512]   : vv[p, jc, e] = v[128*jc+p, e]
  wT  [128, 8, 512]   : wT[p, jc, i] = exp(s*SCALE)[i, 128*jc+p]
  oT  [128, 16, 512]  : oT[p, h, i]  = attn_out[i, 128*h+p]
  y = oT.T @ Wo accumulated over heads, streamed in 512-col blocks,
  written fp16 and upcast host-side.
"""

import numpy as np

T = 4096
D = 2048
H = 16
KVH = 4
HD = 128
WINDOW = 512
SCALE = HD ** -0.5
N_CORES = 8
TLOC = T // N_CORES          # 512 own query rows / core
XROWS = TLOC + WINDOW        # 1024 x rows / core (halo + own)
NT = TLOC // 128             # 4 q-tiles of 128 rows
NJC = XROWS // 128           # 8 key chunks of 128
BAND = WINDOW + 128          # 640 key columns per q-tile
DC = D // 128                # 16 d-chunks
N_REP = H // KVH
MASK_VAL = -1e9

_CACHE = {}


def _emit(nc, tc, tile, mybir, make_identity, loop_n=None, stop_after=None):
    f32 = mybir.dt.float32
    f16 = mybir.dt.float16

    timing = loop_n is not None
    kin = "Internal" if timing else "ExternalInput"
    kout = "Internal" if timing else "ExternalOutput"
    xt_d = nc.dram_tensor("xt", [128, DC * XROWS], f16, kind=kin)
    wq_d = nc.dram_tensor("wq", [128, H * DC * 128], f16, kind=kin)
    wk_d = nc.dram_tensor("wk", [128, KVH * DC * 128], f16, kind=kin)
    wv_d = nc.dram_tensor("wv", [128, DC * 512], f16, kind=kin)
    wo_d = nc.dram_tensor("wo", [128, 4 * H * 512], f16, kind=kin)
    lcorr_d = nc.dram_tensor("lcorr", [128, NT], f32, kind=kin)
    y_d = nc.dram_tensor("y", [TLOC, D], f16, kind=kout)
    if timing:
        dummy_d = nc.dram_tensor("bench_done", [1, 128], f32,
                                 kind="ExternalOutput")

    def mm(out, lhsT, rhs, start, stop):
        nc.tensor.matmul(out, lhsT, rhs, start=start, stop=stop)

    # --- long-lived pools / loop-invariant tiles ---
    # PSUM budget (8 banks): ps_s 2 bufs x [128,640->1024]f32 = 4 banks,
    # ps_ot 2 bufs x [128,512]f32 = 2 banks, plus one phase-scoped
    # right-side pool of <=2 banks (ps_acc in P1, ps_l [l + rT] in P2,
    # ps_acc2 in P3).
    pers = tc.alloc_tile_pool(name="pers", bufs=1)
    ps_s = tc.alloc_tile_pool(name="ps_s", bufs=2, space="PSUM")
    ps_ot = tc.alloc_tile_pool(name="ps_ot", bufs=2, space="PSUM")

    ident = pers.tile([128, 128], f16, tag="ident")
    make_identity(nc, ident[:])
    ones = pers.tile([128, 1], f16, tag="ones")
    nc.gpsimd.memset(ones[:], 1.0)
    # additive edge masks, stacked [128, 2, 128], TRANSPOSED orientation
    # (partition p = key jj, free col = query ii): block 0 = co=0 edge
    # (allowed jj >= ii, keep p >= col), block 1 = co=4 edge (allowed
    # jj <= ii, keep col >= p). Applied as ONE strided DVE add per
    # chunk pair.
    masks = pers.tile([128, 2, 128], f32, tag="masks")
    nc.gpsimd.memset(masks[:], 0.0)
    nc.gpsimd.affine_select(
        out=masks[:, 0, :], in_=masks[:, 0, :],
        compare_op=mybir.AluOpType.is_ge,
        fill=MASK_VAL, base=0, pattern=[[-1, 128]], channel_multiplier=1)
    nc.gpsimd.affine_select(
        out=masks[:, 1, :], in_=masks[:, 1, :],
        compare_op=mybir.AluOpType.is_ge,
        fill=MASK_VAL, base=0, pattern=[[1, 128]], channel_multiplier=-1)

    lp = tc.For_i(0, loop_n, 1) if timing else None
    if lp is not None:
        lp.__enter__()

    proj = tc.alloc_tile_pool(name="proj", bufs=1)
    xp = tc.alloc_tile_pool(name="xp", bufs=1)
    wp = tc.alloc_tile_pool(name="wpool", bufs=2)
    ps_acc = tc.alloc_tile_pool(name="ps_acc", bufs=2, space="PSUM",
                                side="right")

    qT = proj.tile([128, H, TLOC], f16, tag="qT")
    kT = proj.tile([128, KVH, XROWS], f16, tag="kT")
    vv = proj.tile([128, NJC, KVH * HD], f16, tag="vv")
    lcorr_s = proj.tile([128, NT], f32, tag="lcorr")
    xt = xp.tile([128, DC, XROWS], f16, tag="xt")

    nc.sync.dma_start(lcorr_s[:], lcorr_d.ap())

    # ---------------- P1a: k projections (x streamed in) -------------
    # The prologue is DMA-bound (serial transfer resource), so k-proj
    # runs dc-OUTER with all 8 (kv-head, half) accumulation groups open
    # at once across all 8 PSUM banks: each arriving x chunk is fully
    # consumed (8 x 512-col matmuls) before the next chunk lands.
    # DMA order = consumption order: wk g0/g1, x chunks, wk g2/g3, wv.
    wkgs = []
    for g in range(KVH):
        wkg = wp.tile([128, DC, 128], f16, tag="wlhs", name=f"wkg{g}",
                      bufs=4)
        wkgs.append(wkg)
    # first dc-slice of wk g0 split out so the very first matmul can
    # start ~1.5us earlier on the serial DMA stream
    nc.sync.dma_start(wkgs[0][:, 0, :], wk_d.ap()[:, 0:128])
    nc.sync.dma_start(xt[:, 0, :], xt_d.ap()[:, 0:XROWS])
    nc.sync.dma_start(wkgs[0][:, 1:DC, :], wk_d.ap()[:, 128:DC * 128])
    for dc in range(1, DC):
        nc.sync.dma_start(xt[:, dc, :],
                          xt_d.ap()[:, dc * XROWS:(dc + 1) * XROWS])
        if dc == 1:
            # g1 weights after the first x chunks: g0's matmuls cover
            # the PE meanwhile
            nc.sync.dma_start(wkgs[1][:],
                              wk_d.ap()[:, DC * 128:2 * DC * 128])
    nc.sync.dma_start(wkgs[2][:], wk_d.ap()[:, 2 * DC * 128:3 * DC * 128])
    nc.sync.dma_start(wkgs[3][:], wk_d.ap()[:, 3 * DC * 128:4 * DC * 128])

    def kproj_pair(g0, g1, slots, warmup=0):
        """dc-outer over two kv heads: 4 open accumulation groups;
        each x chunk fully consumed on arrival (4 x 512-col matmuls ~
        one chunk's DMA time). warmup: emit g0's first `warmup` chunks
        before g1's so the in-order PE queue isn't blocked on g1's
        weight DMA at startup."""
        gs = [g0, g0, g1, g1]
        for dc in range(warmup):
            for s in range(2):
                mm(slots[s], wkgs[gs[s]][:, dc, :],
                   xt[:, dc, (s % 2) * 512:(s % 2 + 1) * 512],
                   start=(dc == 0), stop=(dc == DC - 1))
        for dc in range(warmup):
            for s in range(2, 4):
                mm(slots[s], wkgs[gs[s]][:, dc, :],
                   xt[:, dc, (s % 2) * 512:(s % 2 + 1) * 512],
                   start=(dc == 0), stop=(dc == DC - 1))
        for dc in range(warmup, DC):
            for s in range(4):
                mm(slots[s], wkgs[gs[s]][:, dc, :],
                   xt[:, dc, (s % 2) * 512:(s % 2 + 1) * 512],
                   start=(dc == 0), stop=(dc == DC - 1))
        for s in range(4):
            # GPSIMD cannot access PSUM on HW: copies go DVE/ACT only
            dst = kT[:, gs[s], (s % 2) * 512:(s % 2 + 1) * 512]
            if s % 2 == 0:
                nc.vector.tensor_copy(dst, slots[s])
            else:
                nc.scalar.copy(dst, slots[s])

    # phase A in 2x[128,1024] ps_s tiles; phase B in ps_ot/ps_acc slots
    # so it does not wait on phase A's PSUM->SBUF copies.
    pkA = [ps_s.tile([128, 1024], f32, tag="score", name=f"pkA{i}")
           for i in range(2)]
    kproj_pair(0, 1, [pkA[0][:, 0:512], pkA[0][:, 512:1024],
                      pkA[1][:, 0:512], pkA[1][:, 512:1024]], warmup=3)
    pkO = [ps_ot.tile([128, TLOC], f32, tag="ot", name=f"pkO{i}")
           for i in range(2)]
    pkB = [ps_acc.tile([128, 512], f32, tag="acc", name=f"pkB{i}")
           for i in range(2)]
    kproj_pair(2, 3, [pkO[0][:], pkO[1][:], pkB[0][:], pkB[1][:]])

    # ---------------- P1b: v projections ------------------------------
    # chunk 7 is deferred into head-0's iteration as PE filler (the
    # pipelined head loop has no PV/rT work for h=0 yet).
    wvt = wp.tile([128, DC, 512], f16, tag="wv", name="wvt", bufs=1)
    nc.sync.dma_start(wvt[:], wv_d.ap())
    for jc in range(NJC - 1):
        pv = ps_acc.tile([128, 512], f32, tag="acc")
        for dc in range(DC):
            mm(pv[:], xt[:, dc, jc * 128:(jc + 1) * 128], wvt[:, dc, :],
               start=(dc == 0), stop=(dc == DC - 1))
        nc.vector.tensor_copy(vv[:, jc, :], pv[:])

    ps_acc.release()

    if stop_after == "kv":
        if lp is not None:
            lp.__exit__(None, None, None)
            dtile = pers.tile([128, 128], f32, tag="dtile")
            nc.vector.memset(dtile[:], 0.0)
            nc.sync.dma_start(dummy_d.ap(), dtile[0:1, :])
        wp.release()
        xp.release()
        proj.release()
        ps_ot.release()
        ps_s.release()
        pers.release()
        return

    # ---------------- P2: attention, q projection interleaved ---------
    attn = tc.alloc_tile_pool(name="attn", bufs=1, side="right")
    # Wo stream pool allocated before sm/ps_l (right-side pools are
    # released in stack order) so the first two 2 MiB chunks can be
    # prefetched during the attention tail.
    wop = tc.alloc_tile_pool(name="wo_pool", bufs=2, side="right")
    sm = tc.alloc_tile_pool(name="sm", bufs=2, side="right")
    ps_l = tc.alloc_tile_pool(name="ps_l", bufs=1, space="PSUM",
                              side="right")

    oT = attn.tile([128, H, TLOC], f16, tag="oT")
    # wT double-buffered by head parity: exps of head h write wTs[h%2]
    # while PV of head h-1 still reads wTs[(h-1)%2].
    wTs = [attn.tile([128, NJC, TLOC], f16, tag=f"wT{i}", name=f"wT{i}")
           for i in range(2)]

    woc_tiles = {}

    def wo_load(dblk):
        woc = wop.tile([128, H, 512], f16, tag="wo", name=f"wo{dblk}")
        nc.sync.dma_start(
            woc[:], wo_d.ap()[:, dblk * H * 512:(dblk + 1) * H * 512])
        woc_tiles[dblk] = woc

    # --- per-head emission pieces (closures over head state) ---------

    def make_scores_T(h):
        """Transposed-score emitters for head h. pair(pi) computes the
        sT blocks of chunks (pi, pi+4) in one [128,1024] PSUM tile
        (2 matmuls streaming qT columns), applies both edge masks with
        one strided DVE add, and exps into wT[:, jc, .] directly.
        lsum(t) accumulates the masked row sums via 5 N=1 matmuls.
        fin() adds lcorr and takes the reciprocal."""
        g = h // N_REP
        wT = wTs[h % 2]
        l_ps = ps_l.tile([128, NT], f32, tag="l", name=f"l{h}", bufs=1)

        def pair(pi):
            jc0, jc1 = pi, pi + 4
            n0, n1 = pi + 1, 4 - pi
            ps = ps_s.tile([128, 1024], f32, tag="score",
                           name=f"sT{h}_{pi}")
            # chunk jc0 covers q-tiles 0..pi at cols [0, n0*128);
            # chunk jc1 covers q-tiles pi..3 at cols [512, 512+n1*128)
            mm(ps[:, 0:n0 * 128], kT[:, g, jc0 * 128:(jc0 + 1) * 128],
               qT[:, h, 0:n0 * 128], start=True, stop=True)
            mm(ps[:, 512:512 + n1 * 128],
               kT[:, g, jc1 * 128:(jc1 + 1) * 128],
               qT[:, h, pi * 128:TLOC], start=True, stop=True)
            # both edge blocks (at cols pi*128 and 512) in one strided
            # DVE add of the stacked masks tile
            s = 512 - pi * 128
            edges = ps[:, pi * 128:pi * 128 + 2 * s].rearrange(
                "p (a b) -> p a b", a=2)[:, :, 0:128]
            nc.vector.tensor_add(edges, edges, masks[:])
            nc.scalar.activation(wT[:, jc0, 0:n0 * 128], ps[:, 0:n0 * 128],
                                 mybir.ActivationFunctionType.Exp,
                                 scale=SCALE)
            nc.scalar.activation(wT[:, jc1, pi * 128:TLOC],
                                 ps[:, 512:512 + n1 * 128],
                                 mybir.ActivationFunctionType.Exp,
                                 scale=SCALE)

        def lsum(t):
            # row sums l[i] for q-tile t: 5 one-column matmuls
            # (stationary = wT chunk, moving = ones) accumulating into
            # l_ps[:, t]. Stationary loads are pipelined; the charged
            # stream is 1 column per matmul.
            for i, jc in enumerate(range(t, t + 5)):
                mm(l_ps[:, t:t + 1], wT[:, jc, t * 128:(t + 1) * 128],
                   ones[:], start=(i == 0), stop=(i == 4))

        def fin():
            # emitted at the START of iteration h+1: by then the
            # l-matmuls have retired, so these DVE ops never block the
            # in-order DVE queue (emitting them right after lsum made
            # the next head's mask adds wait on this head's PE tail).
            lf = sm.tile([128, NT], f32, tag="lf", name=f"lf{h}", bufs=2)
            r = sm.tile([128, NT], f16, tag="r", name=f"r{h}", bufs=2)
            nc.vector.tensor_add(lf[:], l_ps[:], lcorr_s[:])
            with nc.allow_low_precision(reason="1/l fits f16"):
                nc.vector.reciprocal(r[:], lf[:])
            return r

        return pair, lsum, fin, (h, wT)

    def emit_rchain(h, r):
        """r [128, NT] f16 -> flat [1, NT*128] row on partition 0 (4 PE
        column transposes) -> SBUF -> [128, NT*128] partition-broadcast
        tiles (Pool) for the PV-output multiply. The flat layout keeps
        every broadcast input at partition 0 (a BIR requirement)."""
        rT_ps = ps_l.tile([1, NT * 128], f16, tag="rT", name=f"rT{h}",
                          bufs=1)
        for t in range(NT):
            nc.tensor.transpose(rT_ps[0:1, t * 128:(t + 1) * 128],
                                r[:, t:t + 1], ident[:])
        rTs = sm.tile([1, NT * 128], f16, tag="rTs", name=f"rTs{h}",
                      bufs=2)
        nc.vector.tensor_copy(rTs[:], rT_ps[:])
        rbc = sm.tile([128, NT, 128], f16, tag="rbc", name=f"rbc{h}",
                      bufs=2)
        for t in range(NT):
            nc.gpsimd.partition_broadcast(
                rbc[:, t, :], rTs[0:1, t * 128:(t + 1) * 128])
        return rbc

    def make_qp(hq):
        """q-projection matmul chunks for head hq (PE filler)."""
        if hq >= H:
            return (lambda lo, hi: None), (lambda: None)
        wqh = wp.tile([128, DC, 128], f16, tag="wlhs", name=f"wqh{hq}",
                      bufs=4)
        nc.sync.dma_start(
            wqh[:], wq_d.ap()[:, hq * DC * 128:(hq + 1) * DC * 128])
        pq = ps_ot.tile([128, TLOC], f32, tag="ot", name=f"pq{hq}")

        def qp_mms(dc_lo, dc_hi):
            for dc in range(dc_lo, dc_hi):
                mm(pq[:], wqh[:, dc, :], xt[:, dc, WINDOW:XROWS],
                   start=(dc == 0), stop=(dc == DC - 1))

        def qp_fin():
            # (GPSIMD cannot access PSUM on HW)
            nc.scalar.copy(qT[:, hq, :], pq[:])

        return qp_mms, qp_fin

    def make_pv(state):
        """banded PV emitters for head h, reading wTs[h%2] (whose exps
        completed last iteration) + the normalizing oT multiply."""
        h, wT = state
        g = h // N_REP
        po = ps_ot.tile([128, TLOC], f32, tag="ot", name=f"po{h}")

        def pv(t):
            # one accumulation group open at a time per PSUM bank
            for i, jc in enumerate(range(t, t + 5)):
                mm(po[:, t * 128:(t + 1) * 128],
                   vv[:, jc, g * 128:(g + 1) * 128],
                   wT[:, jc, t * 128:(t + 1) * 128],
                   start=(i == 0), stop=(i == 4))

        def ot_fin(rbc):
            # oT[:, h, :] = po * r_i in ONE DVE multiply (the PSUM read
            # that used to be a plain copy now also normalizes)
            nc.vector.tensor_mul(
                oT[:, h, :], po[:],
                rbc[:].rearrange("p a b -> p (a b)"))

        return pv, ot_fin

    py_tiles = {}  # t -> (tile, col half)

    def py_open(dblk):
        pyA = ps_s.tile([128, 1024], f32, tag="score", name=f"pyA{dblk}")
        pyB = ps_s.tile([128, 1024], f32, tag="score", name=f"pyB{dblk}")
        for t in range(NT):
            py_tiles[t] = (pyA if t < 2 else pyB, t % 2)

    def py_mms(t, h_lo, h_hi, dblk):
        woc = woc_tiles[dblk]
        py, half = py_tiles[t]
        for h2 in range(h_lo, h_hi):
            mm(py[:, half * 512:(half + 1) * 512],
               oT[:, h2, t * 128:(t + 1) * 128], woc[:, h2, :],
               start=(h2 == 0), stop=(h2 == H - 1))

    skip_p3 = stop_after == "attn"

    def PY(t, h_lo, h_hi):
        if not skip_p3:
            py_mms(t, h_lo, h_hi, 0)

    def vv_late(jc, eng):
        """deferred v-projection chunk (PE filler in iteration 0)"""
        pvv = ps_ot.tile([128, TLOC], f32, tag="ot", name=f"pvv{jc}")
        for dc in range(DC):
            mm(pvv[:], xt[:, dc, jc * 128:(jc + 1) * 128],
               wvt[:, dc, :], start=(dc == 0), stop=(dc == DC - 1))
        if eng == 0:
            nc.vector.tensor_copy(vv[:, jc, :], pvv[:])
        else:
            nc.scalar.copy(vv[:, jc, :], pvv[:])

    # --- pipelined head loop -----------------------------------------
    # Iteration h emits: scoresT+exp of head h, q-proj of h+1, PV and
    # normalized oT copy-out of h-1, r-chain of h-1 (transpose early so
    # the Pool broadcasts finish before ot_fin needs them).

    # head 0's q-projection runs standalone (heads h+1 ride iteration h)
    qp0_mms, qp0_fin = make_qp(0)
    qp0_mms(0, DC)
    qp0_fin()

    prev = None   # ((h, wT), fin) for head h-1 (PV + normalize stage)
    for h in range(H):
        pair, lsum, fin, cur_state = make_scores_T(h)
        qp_mms, qp_fin = make_qp(h + 1)
        if prev is not None:
            pstate, pfin = prev
            pv, ot_fin = make_pv(pstate)
            pr = pfin()   # DVE lf/recip of h-1; deps already retired
        else:
            pv = ot_fin = None

        def PV(t):
            if pv is not None:
                pv(t)

        pair(0)
        PV(0)
        if prev is not None:
            rbc = emit_rchain(pstate[0], pr)
        pair(1)
        PV(1)
        pair(2)
        PV(2)
        pair(3)
        PV(3)
        if h == 0:
            vv_late(7, 1)
        if h == H - 1 and not skip_p3:
            # no q-projection filler for a 17th head: use Wo block 0's
            # first partial accumulations instead (pyA/pyB bind to the
            # score buffers freed by this head's own exps)
            py_open(0)
            PY(0, 0, 7)
        else:
            qp_mms(0, 8)
        if ot_fin is not None:
            ot_fin(rbc)
        qp_mms(8, DC)
        for t in range(NT):
            lsum(t)
        qp_fin()
        if h == 12:
            wo_load(0)
        elif h == 14:
            wo_load(1)
        prev = (cur_state, fin)

    # --- drain: PV + normalize of head 15, with the first Wo block's
    # partial accumulations (heads 0..13) as PE filler.
    pstate, pfin = prev
    pv15, ot_fin15 = make_pv(pstate)
    pr15 = pfin()
    pv15(0)
    rbc15 = emit_rchain(pstate[0], pr15)
    PY(0, 7, 14)
    pv15(1)
    PY(1, 0, 7)
    pv15(2)
    PY(1, 7, 14)
    pv15(3)
    PY(2, 0, 7)
    ot_fin15(rbc15)
    PY(2, 7, 14)
    PY(3, 0, 7)
    PY(3, 7, 14)

    sm.release()
    ps_l.release()
    wp.release()
    xp.release()
    proj.release()

    if stop_after == "attn":
        if lp is not None:
            lp.__exit__(None, None, None)
            dtile = pers.tile([128, 128], f32, tag="dtile")
            nc.vector.memset(dtile[:], 0.0)
            nc.sync.dma_start(dummy_d.ap(), dtile[0:1, :])
        wop.release()
        attn.release()
        ps_ot.release()
        ps_s.release()
        pers.release()
        return

    # ---------------- P3: output projection ----------------
    # dblk 0's heads 0..13 already accumulated during the drain above;
    # finish with heads 14/15, then stream the remaining Wo blocks.
    def y_out(t, dblk, py_ap):
        ych = attn.tile([128, 512], f16, tag="ych", bufs=3)
        nc.vector.tensor_copy(ych[:], py_ap)
        nc.scalar.dma_start(
            y_d.ap()[t * 128:(t + 1) * 128,
                     dblk * 512:(dblk + 1) * 512],
            ych[:])

    # all h=14 contributions first (oT14 ready early), so the PE has
    # work while the normalized oT15 multiply drains
    for t in range(NT):
        py_mms(t, H - 2, H - 1, 0)
    for t in range(NT):
        py_mms(t, H - 1, H, 0)
        py, half = py_tiles[t]
        y_out(t, 0, py[:, half * 512:(half + 1) * 512])

    for dblk in range(1, 4):
        if dblk not in woc_tiles:
            wo_load(dblk)
        woc = woc_tiles[dblk]
        for t in range(NT):
            # alternate PSUM pools between dblks so a block's first
            # matmuls never wait on the previous block's output copies
            if dblk % 2 == 0:
                py = ps_s.tile([128, 1024], f32, tag="score",
                               name=f"py{dblk}_{t}")
                py_ap = py[:, 0:512]
            else:
                py = ps_ot.tile([128, TLOC], f32, tag="ot",
                                name=f"py{dblk}_{t}")
                py_ap = py[:]
            for h in range(H):
                mm(py_ap, oT[:, h, t * 128:(t + 1) * 128],
                   woc[:, h, :], start=(h == 0), stop=(h == H - 1))
            y_out(t, dblk, py_ap)

    wop.release()
    attn.release()

    if lp is not None:
        lp.__exit__(None, None, None)
        dtile = pers.tile([128, 128], f32, tag="dtile")
        nc.vector.memset(dtile[:], 0.0)
        nc.sync.dma_start(dummy_d.ap(), dtile[0:1, :])

    ps_ot.release()
    ps_s.release()
    pers.release()


def build_nc(loop_n=None, stop_after=None):
    key = ("nc", loop_n, stop_after)
    if key in _CACHE:
        return _CACHE[key]
    import concourse.bacc as bacc
    import concourse.mybir as mybir
    import concourse.tile as tile
    from concourse.masks import make_identity

    nc = bacc.Bacc("TRN2", target_bir_lowering=False, debug=False,
                   num_devices=N_CORES)
    with tile.TileContext(nc) as tc:
        _emit(nc, tc, tile, mybir, make_identity, loop_n=loop_n,
              stop_after=stop_after)
    nc.compile()
    _CACHE[key] = nc
    return nc


def make_inputs_for_core(c, xf, Wq, Wk, Wv, Wo):
    """xf: [T, D] float32 (already squeezed)."""
    f16 = np.float16
    if c == 0:
        x_c = np.concatenate(
            [np.zeros((WINDOW, D), np.float32), xf[:TLOC]], axis=0)
    else:
        x_c = xf[TLOC * c - WINDOW: TLOC * c + TLOC]

    # xt[p, dc, j] = x_c[j, 128*dc+p]
    xt = np.ascontiguousarray(
        x_c.reshape(XROWS, DC, 128).transpose(2, 1, 0).astype(f16)
    ).reshape(128, DC * XROWS)
    # wq[p, h, dc, e] = Wq[128*dc+p, 128*h+e]
    wq = np.ascontiguousarray(
        Wq.reshape(DC, 128, H, 128).transpose(1, 2, 0, 3).astype(f16)
    ).reshape(128, H * DC * 128)
    # wk[p, g, dc, e] = Wk[128*dc+p, 128*g+e]
    wk = np.ascontiguousarray(
        Wk.reshape(DC, 128, KVH, 128).transpose(1, 2, 0, 3).astype(f16)
    ).reshape(128, KVH * DC * 128)
    # wv[p, dc, e] = Wv[128*dc+p, e]
    wv = np.ascontiguousarray(
        Wv.reshape(DC, 128, KVH * HD).transpose(1, 0, 2).astype(f16)
    ).reshape(128, DC * 512)
    # wo[p, dblk, h, e] = Wo[128*h+p, 512*dblk+e]
    wo = np.ascontiguousarray(
        Wo.reshape(H, 128, 4, 512).transpose(1, 2, 0, 3).astype(f16)
    ).reshape(128, 4 * H * 512)

    # core 0: rows see (512 - i) spurious zero-halo keys, each exp(0)=1
    lcorr = np.zeros((128, NT), np.float32)
    if c == 0:
        p = np.arange(128)[:, None]
        t = np.arange(NT)[None, :]
        lcorr = -np.maximum(0, (512 - 128 * t) - p).astype(np.float32)

    return {
        "xt": xt,
        "wq": wq,
        "wk": wk,
        "wv": wv,
        "wo": wo,
        "lcorr": np.ascontiguousarray(lcorr),
    }


def kernel(x, Wq, Wk, Wv, Wo):
    from concourse.bass_utils import run_bass_kernel_spmd

    nc = build_nc()
    xf = np.asarray(x, np.float32).reshape(T, D)
    Wq = np.asarray(Wq, np.float32)
    Wk = np.asarray(Wk, np.float32)
    Wv = np.asarray(Wv, np.float32)
    Wo = np.asarray(Wo, np.float32)
    in_maps = [make_inputs_for_core(c, xf, Wq, Wk, Wv, Wo)
               for c in range(N_CORES)]
    res = run_bass_kernel_spmd(nc, in_maps, core_ids=list(range(N_CORES)))
    y = np.concatenate(
        [res.results[c]["y"].astype(np.float32) for c in range(N_CORES)],
        axis=0)
    return y.reshape(1, T, D)


# revision 8
# speedup vs baseline: 1.0344x; 1.0153x over previous
"""Trainium2 Bass kernel: GQA sliding-window attention (v3, fp16).

Problem: B=1, T=4096, D=2048, H=16 q-heads, KVH=4 kv-heads, HD=128,
causal sliding window 512.

Sharding: 8-way sequence parallel. Core c owns query rows
[512c, 512c+512). It receives x rows [512(c-1), 512(c+1)) (halo of 512
rows; core 0's halo is zeros). Weights replicated. Outputs are disjoint
row blocks -> plain concatenation, no collectives.

v3 changes vs v2: the PE transpose stage is gone. Scores are computed
TRANSPOSED (sT[j,i] = k_j.q_i via stationary kT chunks streaming qT
columns -- same PE cost as forward scores since both q/k live in
[hd-partition, rows] layout), so exp writes wT straight to SBUF in the
exact layout PV consumes. What the forward orientation used to give for
free (row sums via exp accum_out) is recovered with per-tile N=1
matmuls (out [128,1] = wT_chunk.T @ ones): 20 one-column matmuls per
head whose stationary loads the cost model does not charge. The 1/l
normalization moves to the PV output: r = 1/(l+lcorr) is PE-transposed
[128,4]->[4,128], partition-broadcast on Pool to [128,128] tiles, and
applied as one DVE multiply on the PV PSUM during the oT copy-out.

Per-head PE work drops from 7680 cols (scores 2560 + transposes 2560 +
PV 2560) to ~5250 (scoresT 2560 + PV 2560 + rT 128 + l ~0): -38k
charged columns overall. DVE drops the 59us of pt->wT copies; ACT
loses accum_out but keeps the same exp element count.

Transposed-score mask: band block (jc, t) with co = jc - t needs no
mask for co in 1..3; co=0 needs allowed jj>=ii (same triangle as v2's
maskL), co=4 needs jj<=ii (v2's maskR). Chunks are paired (jc, jc+4)
in one [128,1024] PSUM tile (block counts 1+4, 2+3, 3+2, 4+1 -- always
640 cols), with both edge blocks hit by ONE strided DVE add of the
same stacked [128,2,128] mask tile as v2.

Per-core layouts (SBUF partition dim first, all fp16 except f32 sums):
  xt  [128, 16, 1024] : xt[p, dc, j] = x_c[j, 128*dc+p]   (host prep)
  qT  [128, 16, 512]  : qT[p, h, i]  = q[i, 128*h+p]  (unscaled)
  kT  [128, 4, 1024]  : kT[p, g, j]  = k[j, 128*g+p]
  vv  [128, 8, 512]   : vv[p, jc, e] = v[128*jc+p, e]
  wT  [128, 8, 512]   : wT[p, jc, i] = exp(s*SCALE)[i, 128*jc+p]
  oT  [128, 16, 512]  : oT[p, h, i]  = attn_out[i, 128*h+p]
  y = oT.T @ Wo accumulated over heads, streamed in 512-col blocks,
  written fp16 and upcast host-side.
"""

import numpy as np

T = 4096
D = 2048
H = 16
KVH = 4
HD = 128
WINDOW = 512
SCALE = HD ** -0.5
N_CORES = 8
TLOC = T // N_CORES          # 512 own query rows / core
XROWS = TLOC + WINDOW        # 1024 x rows / core (halo + own)
NT = TLOC // 128             # 4 q-tiles of 128 rows
NJC = XROWS // 128           # 8 key chunks of 128
BAND = WINDOW + 128          # 640 key columns per q-tile
DC = D // 128                # 16 d-chunks
N_REP = H // KVH
MASK_VAL = -1e9

_CACHE = {}


def _emit(nc, tc, tile, mybir, make_identity, loop_n=None, stop_after=None):
    f32 = mybir.dt.float32
    f16 = mybir.dt.float16

    timing = loop_n is not None
    kin = "Internal" if timing else "ExternalInput"
    kout = "Internal" if timing else "ExternalOutput"
    xt_d = nc.dram_tensor("xt", [128, DC * XROWS], f16, kind=kin)
    wq_d = nc.dram_tensor("wq", [128, H * DC * 128], f16, kind=kin)
    wk_d = nc.dram_tensor("wk", [128, KVH * DC * 128], f16, kind=kin)
    wv_d = nc.dram_tensor("wv", [128, DC * 512], f16, kind=kin)
    wo_d = nc.dram_tensor("wo", [128, 4 * H * 512], f16, kind=kin)
    lcorr_d = nc.dram_tensor("lcorr", [128, NT], f32, kind=kin)
    y_d = nc.dram_tensor("y", [TLOC, D], f16, kind=kout)
    if timing:
        dummy_d = nc.dram_tensor("bench_done", [1, 128], f32,
                                 kind="ExternalOutput")

    def mm(out, lhsT, rhs, start, stop):
        nc.tensor.matmul(out, lhsT, rhs, start=start, stop=stop)

    # --- long-lived pools / loop-invariant tiles ---
    # PSUM budget (8 banks): ps_s 2 bufs x [128,640->1024]f32 = 4 banks,
    # ps_ot 2 bufs x [128,512]f32 = 2 banks, plus one phase-scoped
    # right-side pool of <=2 banks (ps_acc in P1, ps_l [l + rT] in P2,
    # ps_acc2 in P3).
    pers = tc.alloc_tile_pool(name="pers", bufs=1)
    ps_s = tc.alloc_tile_pool(name="ps_s", bufs=2, space="PSUM")
    ps_ot = tc.alloc_tile_pool(name="ps_ot", bufs=2, space="PSUM")

    ident = pers.tile([128, 128], f16, tag="ident")
    make_identity(nc, ident[:])
    ones = pers.tile([128, 1], f16, tag="ones")
    nc.gpsimd.memset(ones[:], 1.0)
    # additive edge masks, stacked [128, 2, 128], TRANSPOSED orientation
    # (partition p = key jj, free col = query ii): block 0 = co=0 edge
    # (allowed jj >= ii, keep p >= col), block 1 = co=4 edge (allowed
    # jj <= ii, keep col >= p). Applied as ONE strided DVE add per
    # chunk pair.
    masks = pers.tile([128, 2, 128], f32, tag="masks")
    nc.gpsimd.memset(masks[:], 0.0)
    nc.gpsimd.affine_select(
        out=masks[:, 0, :], in_=masks[:, 0, :],
        compare_op=mybir.AluOpType.is_ge,
        fill=MASK_VAL, base=0, pattern=[[-1, 128]], channel_multiplier=1)
    nc.gpsimd.affine_select(
        out=masks[:, 1, :], in_=masks[:, 1, :],
        compare_op=mybir.AluOpType.is_ge,
        fill=MASK_VAL, base=0, pattern=[[1, 128]], channel_multiplier=-1)

    lp = tc.For_i(0, loop_n, 1) if timing else None
    if lp is not None:
        lp.__enter__()

    proj = tc.alloc_tile_pool(name="proj", bufs=1)
    xp = tc.alloc_tile_pool(name="xp", bufs=1)
    wp = tc.alloc_tile_pool(name="wpool", bufs=2)
    ps_acc = tc.alloc_tile_pool(name="ps_acc", bufs=2, space="PSUM",
                                side="right")

    qT = proj.tile([128, H, TLOC], f16, tag="qT")
    kT = proj.tile([128, KVH, XROWS], f16, tag="kT")
    vv = proj.tile([128, NJC, KVH * HD], f16, tag="vv")
    lcorr_s = proj.tile([128, NT], f32, tag="lcorr")
    xt = xp.tile([128, DC, XROWS], f16, tag="xt")

    nc.sync.dma_start(lcorr_s[:], lcorr_d.ap())

    # ---------------- P1a: k projections (x streamed in) -------------
    # The prologue is DMA-bound (serial transfer resource), so k-proj
    # runs dc-OUTER with all 8 (kv-head, half) accumulation groups open
    # at once across all 8 PSUM banks: each arriving x chunk is fully
    # consumed (8 x 512-col matmuls) before the next chunk lands.
    # DMA order = consumption order: wk g0/g1, x chunks, wk g2/g3, wv.
    wkgs = []
    for g in range(KVH):
        wkg = wp.tile([128, DC, 128], f16, tag="wlhs", name=f"wkg{g}",
                      bufs=4)
        wkgs.append(wkg)
    # first dc-slice of wk g0 split out so the very first matmul can
    # start ~1.5us earlier on the serial DMA stream
    nc.sync.dma_start(wkgs[0][:, 0, :], wk_d.ap()[:, 0:128])
    nc.sync.dma_start(xt[:, 0, :], xt_d.ap()[:, 0:XROWS])
    nc.sync.dma_start(wkgs[0][:, 1:DC, :], wk_d.ap()[:, 128:DC * 128])
    for dc in range(1, DC):
        nc.sync.dma_start(xt[:, dc, :],
                          xt_d.ap()[:, dc * XROWS:(dc + 1) * XROWS])
        if dc == 1:
            # g1 weights after the first x chunks: g0's matmuls cover
            # the PE meanwhile
            nc.sync.dma_start(wkgs[1][:],
                              wk_d.ap()[:, DC * 128:2 * DC * 128])
    nc.sync.dma_start(wkgs[2][:], wk_d.ap()[:, 2 * DC * 128:3 * DC * 128])
    nc.sync.dma_start(wkgs[3][:], wk_d.ap()[:, 3 * DC * 128:4 * DC * 128])

    def kproj_pair(g0, g1, slots, warmup=0):
        """dc-outer over two kv heads: 4 open accumulation groups;
        each x chunk fully consumed on arrival (4 x 512-col matmuls ~
        one chunk's DMA time). warmup: emit g0's first `warmup` chunks
        before g1's so the in-order PE queue isn't blocked on g1's
        weight DMA at startup."""
        gs = [g0, g0, g1, g1]
        for dc in range(warmup):
            for s in range(2):
                mm(slots[s], wkgs[gs[s]][:, dc, :],
                   xt[:, dc, (s % 2) * 512:(s % 2 + 1) * 512],
                   start=(dc == 0), stop=(dc == DC - 1))
        for dc in range(warmup):
            for s in range(2, 4):
                mm(slots[s], wkgs[gs[s]][:, dc, :],
                   xt[:, dc, (s % 2) * 512:(s % 2 + 1) * 512],
                   start=(dc == 0), stop=(dc == DC - 1))
        for dc in range(warmup, DC):
            for s in range(4):
                mm(slots[s], wkgs[gs[s]][:, dc, :],
                   xt[:, dc, (s % 2) * 512:(s % 2 + 1) * 512],
                   start=(dc == 0), stop=(dc == DC - 1))
        for s in range(4):
            # GPSIMD cannot access PSUM on HW: copies go DVE/ACT only
            dst = kT[:, gs[s], (s % 2) * 512:(s % 2 + 1) * 512]
            if s % 2 == 0:
                nc.vector.tensor_copy(dst, slots[s])
            else:
                nc.scalar.copy(dst, slots[s])

    # phase A in 2x[128,1024] ps_s tiles; phase B in ps_ot/ps_acc slots
    # so it does not wait on phase A's PSUM->SBUF copies.
    pkA = [ps_s.tile([128, 1024], f32, tag="score", name=f"pkA{i}")
           for i in range(2)]
    kproj_pair(0, 1, [pkA[0][:, 0:512], pkA[0][:, 512:1024],
                      pkA[1][:, 0:512], pkA[1][:, 512:1024]], warmup=3)
    pkO = [ps_ot.tile([128, TLOC], f32, tag="ot", name=f"pkO{i}")
           for i in range(2)]
    pkB = [ps_acc.tile([128, 512], f32, tag="acc", name=f"pkB{i}")
           for i in range(2)]
    kproj_pair(2, 3, [pkO[0][:], pkO[1][:], pkB[0][:], pkB[1][:]])

    # ---------------- P1b: v projections ------------------------------
    # chunk 7 is deferred into head-0's iteration as PE filler (the
    # pipelined head loop has no PV/rT work for h=0 yet).
    wvt = wp.tile([128, DC, 512], f16, tag="wv", name="wvt", bufs=1)
    nc.sync.dma_start(wvt[:], wv_d.ap())
    for jc in range(NJC - 1):
        pv = ps_acc.tile([128, 512], f32, tag="acc")
        for dc in range(DC):
            mm(pv[:], xt[:, dc, jc * 128:(jc + 1) * 128], wvt[:, dc, :],
               start=(dc == 0), stop=(dc == DC - 1))
        nc.vector.tensor_copy(vv[:, jc, :], pv[:])

    ps_acc.release()

    if stop_after == "kv":
        if lp is not None:
            lp.__exit__(None, None, None)
            dtile = pers.tile([128, 128], f32, tag="dtile")
            nc.vector.memset(dtile[:], 0.0)
            nc.sync.dma_start(dummy_d.ap(), dtile[0:1, :])
        wp.release()
        xp.release()
        proj.release()
        ps_ot.release()
        ps_s.release()
        pers.release()
        return

    # ---------------- P2: attention, q projection interleaved ---------
    attn = tc.alloc_tile_pool(name="attn", bufs=1, side="right")
    # Wo stream pool allocated before sm/ps_l (right-side pools are
    # released in stack order) so the first two 2 MiB chunks can be
    # prefetched during the attention tail.
    wop = tc.alloc_tile_pool(name="wo_pool", bufs=2, side="right")
    sm = tc.alloc_tile_pool(name="sm", bufs=2, side="right")
    ps_l = tc.alloc_tile_pool(name="ps_l", bufs=1, space="PSUM",
                              side="right")

    oT = attn.tile([128, H, TLOC], f16, tag="oT")
    # wT double-buffered by head parity: exps of head h write wTs[h%2]
    # while PV of head h-1 still reads wTs[(h-1)%2].
    wTs = [attn.tile([128, NJC, TLOC], f16, tag=f"wT{i}", name=f"wT{i}")
           for i in range(2)]

    woc_tiles = {}

    def wo_load(dblk):
        woc = wop.tile([128, H, 512], f16, tag="wo", name=f"wo{dblk}")
        nc.sync.dma_start(
            woc[:], wo_d.ap()[:, dblk * H * 512:(dblk + 1) * H * 512])
        woc_tiles[dblk] = woc

    # --- per-head emission pieces (closures over head state) ---------

    def make_scores_T(h):
        """Transposed-score emitters for head h. pair(pi) computes the
        sT blocks of chunks (pi, pi+4) in one [128,1024] PSUM tile
        (2 matmuls streaming qT columns), applies both edge masks with
        one strided DVE add, and exps into wT[:, jc, .] directly.
        lsum(t) accumulates the masked row sums via 5 N=1 matmuls.
        fin() adds lcorr and takes the reciprocal."""
        g = h // N_REP
        wT = wTs[h % 2]
        l_ps = ps_l.tile([128, NT], f32, tag="l", name=f"l{h}", bufs=1)

        def pair(pi):
            jc0, jc1 = pi, pi + 4
            n0, n1 = pi + 1, 4 - pi
            ps = ps_s.tile([128, 1024], f32, tag="score",
                           name=f"sT{h}_{pi}")
            # chunk jc0 covers q-tiles 0..pi at cols [0, n0*128);
            # chunk jc1 covers q-tiles pi..3 at cols [512, 512+n1*128)
            mm(ps[:, 0:n0 * 128], kT[:, g, jc0 * 128:(jc0 + 1) * 128],
               qT[:, h, 0:n0 * 128], start=True, stop=True)
            mm(ps[:, 512:512 + n1 * 128],
               kT[:, g, jc1 * 128:(jc1 + 1) * 128],
               qT[:, h, pi * 128:TLOC], start=True, stop=True)
            # both edge blocks (at cols pi*128 and 512) in one strided
            # DVE add of the stacked masks tile
            s = 512 - pi * 128
            edges = ps[:, pi * 128:pi * 128 + 2 * s].rearrange(
                "p (a b) -> p a b", a=2)[:, :, 0:128]
            nc.vector.tensor_add(edges, edges, masks[:])
            nc.scalar.activation(wT[:, jc0, 0:n0 * 128], ps[:, 0:n0 * 128],
                                 mybir.ActivationFunctionType.Exp,
                                 scale=SCALE)
            nc.scalar.activation(wT[:, jc1, pi * 128:TLOC],
                                 ps[:, 512:512 + n1 * 128],
                                 mybir.ActivationFunctionType.Exp,
                                 scale=SCALE)

        def lsum(t):
            # row sums l[i] for q-tile t: 5 one-column matmuls
            # (stationary = wT chunk, moving = ones) accumulating into
            # l_ps[:, t]. Stationary loads are pipelined; the charged
            # stream is 1 column per matmul.
            for i, jc in enumerate(range(t, t + 5)):
                mm(l_ps[:, t:t + 1], wT[:, jc, t * 128:(t + 1) * 128],
                   ones[:], start=(i == 0), stop=(i == 4))

        def fin():
            # emitted at the START of iteration h+1: by then the
            # l-matmuls have retired, so these DVE ops never block the
            # in-order DVE queue (emitting them right after lsum made
            # the next head's mask adds wait on this head's PE tail).
            lf = sm.tile([128, NT], f32, tag="lf", name=f"lf{h}", bufs=2)
            r = sm.tile([128, NT], f16, tag="r", name=f"r{h}", bufs=2)
            nc.vector.tensor_add(lf[:], l_ps[:], lcorr_s[:])
            with nc.allow_low_precision(reason="1/l fits f16"):
                nc.vector.reciprocal(r[:], lf[:])
            return r

        return pair, lsum, fin, (h, wT)

    def emit_rchain(h, r):
        """r [128, NT] f16 -> flat [1, NT*128] row on partition 0 (4 PE
        column transposes) -> SBUF -> [128, NT*128] partition-broadcast
        tiles (Pool) for the PV-output multiply. The flat layout keeps
        every broadcast input at partition 0 (a BIR requirement)."""
        rT_ps = ps_l.tile([1, NT * 128], f16, tag="rT", name=f"rT{h}",
                          bufs=1)
        for t in range(NT):
            nc.tensor.transpose(rT_ps[0:1, t * 128:(t + 1) * 128],
                                r[:, t:t + 1], ident[:])
        rTs = sm.tile([1, NT * 128], f16, tag="rTs", name=f"rTs{h}",
                      bufs=2)
        nc.vector.tensor_copy(rTs[:], rT_ps[:])
        rbc = sm.tile([128, NT, 128], f16, tag="rbc", name=f"rbc{h}",
                      bufs=2)
        for t in range(NT):
            nc.gpsimd.partition_broadcast(
                rbc[:, t, :], rTs[0:1, t * 128:(t + 1) * 128])
        return rbc

    def make_qp(hq):
        """q-projection matmul chunks for head hq (PE filler)."""
        if hq >= H:
            return (lambda lo, hi: None), (lambda: None)
        wqh = wp.tile([128, DC, 128], f16, tag="wlhs", name=f"wqh{hq}",
                      bufs=4)
        nc.sync.dma_start(
            wqh[:], wq_d.ap()[:, hq * DC * 128:(hq + 1) * DC * 128])
        pq = ps_ot.tile([128, TLOC], f32, tag="ot", name=f"pq{hq}")

        def qp_mms(dc_lo, dc_hi):
            for dc in range(dc_lo, dc_hi):
                mm(pq[:], wqh[:, dc, :], xt[:, dc, WINDOW:XROWS],
                   start=(dc == 0), stop=(dc == DC - 1))

        def qp_fin():
            # (GPSIMD cannot access PSUM on HW)
            nc.scalar.copy(qT[:, hq, :], pq[:])

        return qp_mms, qp_fin

    def make_pv(state):
        """banded PV emitters for head h, reading wTs[h%2] (whose exps
        completed last iteration) + the normalizing oT multiply."""
        h, wT = state
        g = h // N_REP
        po = ps_ot.tile([128, TLOC], f32, tag="ot", name=f"po{h}")

        def pv(t):
            # one accumulation group open at a time per PSUM bank
            for i, jc in enumerate(range(t, t + 5)):
                mm(po[:, t * 128:(t + 1) * 128],
                   vv[:, jc, g * 128:(g + 1) * 128],
                   wT[:, jc, t * 128:(t + 1) * 128],
                   start=(i == 0), stop=(i == 4))

        def ot_fin(rbc):
            # oT[:, h, :] = po * r_i in ONE DVE multiply (the PSUM read
            # that used to be a plain copy now also normalizes)
            nc.vector.tensor_mul(
                oT[:, h, :], po[:],
                rbc[:].rearrange("p a b -> p (a b)"))

        return pv, ot_fin

    py_tiles = {}  # t -> (tile, col half)

    def py_open(dblk):
        pyA = ps_s.tile([128, 1024], f32, tag="score", name=f"pyA{dblk}")
        pyB = ps_s.tile([128, 1024], f32, tag="score", name=f"pyB{dblk}")
        for t in range(NT):
            py_tiles[t] = (pyA if t < 2 else pyB, t % 2)

    def py_mms(t, h_lo, h_hi, dblk):
        woc = woc_tiles[dblk]
        py, half = py_tiles[t]
        for h2 in range(h_lo, h_hi):
            mm(py[:, half * 512:(half + 1) * 512],
               oT[:, h2, t * 128:(t + 1) * 128], woc[:, h2, :],
               start=(h2 == 0), stop=(h2 == H - 1))

    skip_p3 = stop_after == "attn"

    def PY(t, h_lo, h_hi):
        if not skip_p3:
            py_mms(t, h_lo, h_hi, 0)

    def vv_late(jc, eng):
        """deferred v-projection chunk (PE filler in iteration 0)"""
        pvv = ps_ot.tile([128, TLOC], f32, tag="ot", name=f"pvv{jc}")
        for dc in range(DC):
            mm(pvv[:], xt[:, dc, jc * 128:(jc + 1) * 128],
               wvt[:, dc, :], start=(dc == 0), stop=(dc == DC - 1))
        if eng == 0:
            nc.vector.tensor_copy(vv[:, jc, :], pvv[:])
        else:
            nc.scalar.copy(vv[:, jc, :], pvv[:])

    # --- pipelined head loop -----------------------------------------
    # Iteration h emits: scoresT+exp of head h, q-proj of h+1, PV and
    # normalized oT copy-out of h-1, r-chain of h-1 (transpose early so
    # the Pool broadcasts finish before ot_fin needs them).

    # head 0's q-projection runs standalone (heads h+1 ride iteration h)
    qp0_mms, qp0_fin = make_qp(0)
    qp0_mms(0, DC)
    qp0_fin()

    prev = None   # ((h, wT), lsum, fin) for head h-1
    for h in range(H):
        pair, lsum, fin, cur_state = make_scores_T(h)
        qp_mms, qp_fin = make_qp(h + 1)
        if prev is not None:
            pstate, plsum, pfin = prev
            pv, ot_fin = make_pv(pstate)
            # l-matmuls + lf/recip of h-1 at iteration START: all their
            # deps (exps of h-1) retired mid-last-iteration, so neither
            # the PE nor the DVE queue ever blocks on them here.
            for t in range(NT):
                plsum(t)
            pr = pfin()
        else:
            pv = ot_fin = None

        def PV(t):
            if pv is not None:
                pv(t)

        pair(0)
        PV(0)
        if prev is not None:
            rbc = emit_rchain(pstate[0], pr)
        pair(1)
        PV(1)
        if h == 0:
            vv_late(7, 1)
        qp_mms(0, 4)
        pair(2)
        PV(2)
        qp_mms(4, 8)
        pair(3)
        PV(3)
        if h == H - 1 and not skip_p3:
            # no q-projection filler for a 17th head: use Wo block 0's
            # first partial accumulations instead (pyA/pyB bind to the
            # score buffers freed by this head's own exps)
            py_open(0)
            PY(0, 0, 7)
        if ot_fin is not None:
            ot_fin(rbc)
        qp_mms(8, DC)
        qp_fin()
        if h == 12:
            wo_load(0)
        elif h == 14:
            wo_load(1)
        prev = (cur_state, lsum, fin)

    # --- drain: l/PV/normalize of head 15, with the first Wo block's
    # partial accumulations (heads 0..13) as PE filler.
    pstate, plsum, pfin = prev
    pv15, ot_fin15 = make_pv(pstate)
    for t in range(NT):
        plsum(t)
    pr15 = pfin()
    pv15(0)
    rbc15 = emit_rchain(pstate[0], pr15)
    PY(0, 7, 14)
    pv15(1)
    PY(1, 0, 7)
    pv15(2)
    PY(1, 7, 14)
    pv15(3)
    PY(2, 0, 7)
    ot_fin15(rbc15)
    PY(2, 7, 14)
    PY(3, 0, 7)
    PY(3, 7, 14)

    sm.release()
    ps_l.release()
    wp.release()
    xp.release()
    proj.release()

    if stop_after == "attn":
        if lp is not None:
            lp.__exit__(None, None, None)
            dtile = pers.tile([128, 128], f32, tag="dtile")
            nc.vector.memset(dtile[:], 0.0)
            nc.sync.dma_start(dummy_d.ap(), dtile[0:1, :])
        wop.release()
        attn.release()
        ps_ot.release()
        ps_s.release()
        pers.release()
        return

    # ---------------- P3: output projection ----------------
    # dblk 0's heads 0..13 already accumulated during the drain above;
    # finish with heads 14/15, then stream the remaining Wo blocks.
    def y_out(t, dblk, py_ap):
        ych = attn.tile([128, 512], f16, tag="ych", bufs=3)
        nc.vector.tensor_copy(ych[:], py_ap)
        nc.scalar.dma_start(
            y_d.ap()[t * 128:(t + 1) * 128,
                     dblk * 512:(dblk + 1) * 512],
            ych[:])

    # all h=14 contributions first (oT14 ready early), so the PE has
    # work while the normalized oT15 multiply drains
    for t in range(NT):
        py_mms(t, H - 2, H - 1, 0)
    for t in range(NT):
        py_mms(t, H - 1, H, 0)
        py, half = py_tiles[t]
        y_out(t, 0, py[:, half * 512:(half + 1) * 512])

    for dblk in range(1, 4):
        if dblk not in woc_tiles:
            wo_load(dblk)
        woc = woc_tiles[dblk]
        for t in range(NT):
            # alternate PSUM pools between dblks so a block's first
            # matmuls never wait on the previous block's output copies
            if dblk % 2 == 0:
                py = ps_s.tile([128, 1024], f32, tag="score",
                               name=f"py{dblk}_{t}")
                py_ap = py[:, 0:512]
            else:
                py = ps_ot.tile([128, TLOC], f32, tag="ot",
                                name=f"py{dblk}_{t}")
                py_ap = py[:]
            for h in range(H):
                mm(py_ap, oT[:, h, t * 128:(t + 1) * 128],
                   woc[:, h, :], start=(h == 0), stop=(h == H - 1))
            y_out(t, dblk, py_ap)

    wop.release()
    attn.release()

    if lp is not None:
        lp.__exit__(None, None, None)
        dtile = pers.tile([128, 128], f32, tag="dtile")
        nc.vector.memset(dtile[:], 0.0)
        nc.sync.dma_start(dummy_d.ap(), dtile[0:1, :])

    ps_ot.release()
    ps_s.release()
    pers.release()


def build_nc(loop_n=None, stop_after=None):
    key = ("nc", loop_n, stop_after)
    if key in _CACHE:
        return _CACHE[key]
    import concourse.bacc as bacc
    import concourse.mybir as mybir
    import concourse.tile as tile
    from concourse.masks import make_identity

    nc = bacc.Bacc("TRN2", target_bir_lowering=False, debug=False,
                   num_devices=N_CORES)
    with tile.TileContext(nc) as tc:
        _emit(nc, tc, tile, mybir, make_identity, loop_n=loop_n,
              stop_after=stop_after)
    nc.compile()
    _CACHE[key] = nc
    return nc


def make_inputs_for_core(c, xf, Wq, Wk, Wv, Wo):
    """xf: [T, D] float32 (already squeezed)."""
    f16 = np.float16
    if c == 0:
        x_c = np.concatenate(
            [np.zeros((WINDOW, D), np.float32), xf[:TLOC]], axis=0)
    else:
        x_c = xf[TLOC * c - WINDOW: TLOC * c + TLOC]

    # xt[p, dc, j] = x_c[j, 128*dc+p]
    xt = np.ascontiguousarray(
        x_c.reshape(XROWS, DC, 128).transpose(2, 1, 0).astype(f16)
    ).reshape(128, DC * XROWS)
    # wq[p, h, dc, e] = Wq[128*dc+p, 128*h+e]
    wq = np.ascontiguousarray(
        Wq.reshape(DC, 128, H, 128).transpose(1, 2, 0, 3).astype(f16)
    ).reshape(128, H * DC * 128)
    # wk[p, g, dc, e] = Wk[128*dc+p, 128*g+e]
    wk = np.ascontiguousarray(
        Wk.reshape(DC, 128, KVH, 128).transpose(1, 2, 0, 3).astype(f16)
    ).reshape(128, KVH * DC * 128)
    # wv[p, dc, e] = Wv[128*dc+p, e]
    wv = np.ascontiguousarray(
        Wv.reshape(DC, 128, KVH * HD).transpose(1, 0, 2).astype(f16)
    ).reshape(128, DC * 512)
    # wo[p, dblk, h, e] = Wo[128*h+p, 512*dblk+e]
    wo = np.ascontiguousarray(
        Wo.reshape(H, 128, 4, 512).transpose(1, 2, 0, 3).astype(f16)
    ).reshape(128, 4 * H * 512)

    # core 0: rows see (512 - i) spurious zero-halo keys, each exp(0)=1
    lcorr = np.zeros((128, NT), np.float32)
    if c == 0:
        p = np.arange(128)[:, None]
        t = np.arange(NT)[None, :]
        lcorr = -np.maximum(0, (512 - 128 * t) - p).astype(np.float32)

    return {
        "xt": xt,
        "wq": wq,
        "wk": wk,
        "wv": wv,
        "wo": wo,
        "lcorr": np.ascontiguousarray(lcorr),
    }


def kernel(x, Wq, Wk, Wv, Wo):
    from concourse.bass_utils import run_bass_kernel_spmd

    nc = build_nc()
    xf = np.asarray(x, np.float32).reshape(T, D)
    Wq = np.asarray(Wq, np.float32)
    Wk = np.asarray(Wk, np.float32)
    Wv = np.asarray(Wv, np.float32)
    Wo = np.asarray(Wo, np.float32)
    in_maps = [make_inputs_for_core(c, xf, Wq, Wk, Wv, Wo)
               for c in range(N_CORES)]
    res = run_bass_kernel_spmd(nc, in_maps, core_ids=list(range(N_CORES)))
    y = np.concatenate(
        [res.results[c]["y"].astype(np.float32) for c in range(N_CORES)],
        axis=0)
    return y.reshape(1, T, D)


# revision 57
# speedup vs baseline: 1.1096x; 1.0727x over previous
"""Trainium2 Bass kernel: GQA sliding-window attention (v3, fp16).

Problem: B=1, T=4096, D=2048, H=16 q-heads, KVH=4 kv-heads, HD=128,
causal sliding window 512.

Sharding: 8-way sequence parallel. Core c owns query rows
[512c, 512c+512). It receives x rows [512(c-1), 512(c+1)) (halo of 512
rows; core 0's halo is zeros). Weights replicated. Outputs are disjoint
row blocks -> plain concatenation, no collectives.

NEXT-STEP ROADMAP (analyzed, not implemented -- needs a full session):
the k/v halo recompute costs 2x32768 charged PE cycles per core. Two
ways to remove it: (a) cross-core remote DMA of own-row k/v to the
right neighbor (bass remote_dma_broadcast; the cost model has RDMA_D2D
constants, but the bass2jax/PJRT execution path is unvalidated); or
(b) reshard as 4 kv-groups x 2 sequence halves: each core computes
k/v for ONE group over 2560 rows (40960+40960 charged vs 65536+65536)
and a PARTIAL y over its group's 512 he-columns, summed at gather
time (the sharding_hint endorses head-parallel, whose unshard is a
reduction). (b) needs no collectives, keeps the same per-strip band
structure (virtual head = (head, 512-row strip), identical 8-chunk
window), and drops the charged floor from 483k to 434k cycles
(~201 -> ~181us), but x/y DMA grows to ~21.5 MiB/core -- the
prologue becomes x-DMA-bound and P3 needs 64 smaller y writes.

v3 changes vs v2 (232793ns -> 219088ns):
  - The PE transpose stage is gone. Scores are computed TRANSPOSED
    (sT[j,i] = k_j.q_i via stationary kT chunks streaming qT columns,
    same PE cost as forward scores since q/k both live in
    [hd-partition, rows] layout), so exp writes wT straight to SBUF in
    the exact layout PV consumes.
  - Row sums (previously free via exp accum_out) come from 20 N=1
    matmuls per head (out [128,1] = wT_chunk.T @ ones), interleaved
    under the 512-col qp chunks so their per-instruction latency hides
    behind engine-busy streaming.
  - The window mask moved OFF the critical matmul->exp chain: exp runs
    on the raw band scores (O(1) values, no overflow), and the two
    edge-block triangles per chunk pair are zeroed afterwards in wT by
    Pool affine_selects (Pool is otherwise idle). This frees the score
    PSUM slots at exp time and removed all per-pair DVE mask adds.
  - PV is oriented [i, e] (stationary = wT chunk, moving = v chunk) so
    the 1/l normalization is one per-partition DVE tensor_scalar
    multiply with r = 1/(l+lcorr); the normalized o tiles are then
    PE-transposed back to oT [e, i] (4x128-col transposes/head) one
    iteration later, where the deps are fully retired.
  - Chunk pairing: (jc, jc+4) share one [128,1024] PSUM tile (block
    counts 1+4, 2+3, 3+2, 4+1 -- always 640 cols).
  - Per-head PE work drops from 7680 charged cols (scores 2560 +
    transposes 2560 + PV 2560) to ~5200 (scoresT 2560 + PV 2560 + oT
    transposes 512 + l ~0).
  - 2-deep software pipeline per head h: scoresT/exp(h), qp(h+1),
    PV/normalize(h-1), oT-transposes(h-2), ordered so the ps_s
    score-slot recycle (pair p waits exp of pair p-2) and all
    cross-engine chains have PE runway in front of them.

Per-core layouts (SBUF partition dim first, all fp16 except f32 sums):
  xt  [128, 16, 1024] : xt[p, dc, j] = x_c[j, 128*dc+p]   (host prep)
  qT  [128, 16, 512]  : qT[p, h, i]  = q[i, 128*h+p]  (unscaled)
  kT  [128, 4, 1024]  : kT[p, g, j]  = k[j, 128*g+p]
  vv  [128, 8, 512]   : vv[p, jc, e] = v[128*jc+p, e]
  wT  [128, 8, 512]   : wT[p, jc, i] = exp(s*SCALE)[i, 128*jc+p]
  oT  [128, 16, 512]  : oT[p, h, i]  = attn_out[i, 128*h+p]
  y = oT.T @ Wo accumulated over heads, streamed in 512-col blocks,
  written fp16 and upcast host-side.
"""

import numpy as np

T = 4096
D = 2048
H = 16
KVH = 4
HD = 128
WINDOW = 512
SCALE = HD ** -0.5
N_CORES = 8
TLOC = T // N_CORES          # 512 own query rows / core
XROWS = TLOC + WINDOW        # 1024 x rows / core (halo + own)
NT = TLOC // 128             # 4 q-tiles of 128 rows
NJC = XROWS // 128           # 8 key chunks of 128
BAND = WINDOW + 128          # 640 key columns per q-tile
DC = D // 128                # 16 d-chunks
N_REP = H // KVH
MASK_VAL = -1e9

_CACHE = {}


def _emit(nc, tc, tile, mybir, make_identity, loop_n=None, stop_after=None):
    f32 = mybir.dt.float32
    f16 = mybir.dt.float16

    timing = loop_n is not None
    kin = "Internal" if timing else "ExternalInput"
    kout = "Internal" if timing else "ExternalOutput"
    xt_d = nc.dram_tensor("xt", [128, DC * XROWS], f16, kind=kin)
    wq_d = nc.dram_tensor("wq", [128, H * DC * 128], f16, kind=kin)
    wk_d = nc.dram_tensor("wk", [128, KVH * DC * 128], f16, kind=kin)
    wv_d = nc.dram_tensor("wv", [128, DC * 512], f16, kind=kin)
    wo_d = nc.dram_tensor("wo", [128, 4 * H * 512], f16, kind=kin)
    lcorr_d = nc.dram_tensor("lcorr", [128, NT], f32, kind=kin)
    y_d = nc.dram_tensor("y", [TLOC, D], f16, kind=kout)
    if timing:
        dummy_d = nc.dram_tensor("bench_done", [1, 128], f32,
                                 kind="ExternalOutput")

    def mm(out, lhsT, rhs, start, stop):
        nc.tensor.matmul(out, lhsT, rhs, start=start, stop=stop)

    # --- long-lived pools / loop-invariant tiles ---
    # PSUM budget (8 banks): ps_s 2 bufs x [128,640->1024]f32 = 4 banks,
    # ps_ot 2 bufs x [128,512]f32 = 2 banks, plus one phase-scoped
    # right-side pool of <=2 banks (ps_acc in P1, ps_l [l + rT] in P2,
    # ps_acc2 in P3).
    pers = tc.alloc_tile_pool(name="pers", bufs=1)
    ps_s = tc.alloc_tile_pool(name="ps_s", bufs=2, space="PSUM")
    ps_ot = tc.alloc_tile_pool(name="ps_ot", bufs=2, space="PSUM")

    ident = pers.tile([128, 128], f16, tag="ident")
    make_identity(nc, ident[:])
    ones = pers.tile([128, 1], f16, tag="ones")
    nc.gpsimd.memset(ones[:], 1.0)

    lp = tc.For_i(0, loop_n, 1) if timing else None
    if lp is not None:
        lp.__enter__()

    proj = tc.alloc_tile_pool(name="proj", bufs=1)
    xp = tc.alloc_tile_pool(name="xp", bufs=1)
    wp = tc.alloc_tile_pool(name="wpool", bufs=2)
    ps_acc = tc.alloc_tile_pool(name="ps_acc", bufs=2, space="PSUM",
                                side="right")

    qT = proj.tile([128, H, TLOC], f16, tag="qT")
    kT = proj.tile([128, KVH, XROWS], f16, tag="kT")
    vv = proj.tile([128, NJC, KVH * HD], f16, tag="vv")
    lcorr_s = proj.tile([128, NT], f32, tag="lcorr")
    xt = xp.tile([128, DC, XROWS], f16, tag="xt")

    # ---------------- P1a: k projections (x streamed in) -------------
    # The prologue is DMA-bound (serial transfer resource), so k-proj
    # runs dc-OUTER with all 8 (kv-head, half) accumulation groups open
    # at once across all 8 PSUM banks: each arriving x chunk is fully
    # consumed (8 x 512-col matmuls) before the next chunk lands.
    # DMA order = consumption order: wk g0/g1, x chunks, wk g2/g3, wv.
    wkgs = []
    for g in range(KVH):
        wkg = wp.tile([128, DC, 128], f16, tag="wlhs", name=f"wkg{g}",
                      bufs=4)
        wkgs.append(wkg)
    # first dc-slice of wk g0 + the first HALF of x chunk 0 split out
    # so the very first matmul waits on the minimum possible bytes of
    # the serial DMA stream (each dma_start also costs a 625ns HWDGE
    # slot, so the critical-path queue must stay short)
    nc.sync.dma_start(wkgs[0][:, 0, :], wk_d.ap()[:, 0:128])
    nc.sync.dma_start(xt[:, 0, :], xt_d.ap()[:, 0:XROWS])
    nc.sync.dma_start(wkgs[0][:, 1:DC, :], wk_d.ap()[:, 128:DC * 128])
    for dc in range(1, DC):
        nc.sync.dma_start(xt[:, dc, :],
                          xt_d.ap()[:, dc * XROWS:(dc + 1) * XROWS])
        if dc == 1:
            # g1 weights after the first x chunks: g0's matmuls cover
            # the PE meanwhile
            nc.sync.dma_start(wkgs[1][:],
                              wk_d.ap()[:, DC * 128:2 * DC * 128])
    nc.sync.dma_start(wkgs[2][:], wk_d.ap()[:, 2 * DC * 128:3 * DC * 128])
    nc.sync.dma_start(wkgs[3][:], wk_d.ap()[:, 3 * DC * 128:4 * DC * 128])
    # lcorr is tiny and first needed mid-head-loop: keep it off the
    # latency-critical head of the DMA queue
    nc.sync.dma_start(lcorr_s[:], lcorr_d.ap())

    def kproj_pair(g0, g1, slots, warmup=0):
        """dc-outer over two kv heads: 4 open accumulation groups;
        each x chunk fully consumed on arrival (4 x 512-col matmuls ~
        one chunk's DMA time). warmup: emit g0's first `warmup` chunks
        before g1's so the in-order PE queue isn't blocked on g1's
        weight DMA at startup."""
        gs = [g0, g0, g1, g1]
        for dc in range(warmup):
            for s in range(2):
                mm(slots[s], wkgs[gs[s]][:, dc, :],
                   xt[:, dc, (s % 2) * 512:(s % 2 + 1) * 512],
                   start=(dc == 0), stop=(dc == DC - 1))
        for dc in range(warmup):
            for s in range(2, 4):
                mm(slots[s], wkgs[gs[s]][:, dc, :],
                   xt[:, dc, (s % 2) * 512:(s % 2 + 1) * 512],
                   start=(dc == 0), stop=(dc == DC - 1))
        for dc in range(warmup, DC):
            for s in range(4):
                mm(slots[s], wkgs[gs[s]][:, dc, :],
                   xt[:, dc, (s % 2) * 512:(s % 2 + 1) * 512],
                   start=(dc == 0), stop=(dc == DC - 1))
        for s in range(4):
            # GPSIMD cannot access PSUM on HW: copies go DVE/ACT only
            dst = kT[:, gs[s], (s % 2) * 512:(s % 2 + 1) * 512]
            if s % 2 == 0:
                nc.vector.tensor_copy(dst, slots[s])
            else:
                nc.scalar.copy(dst, slots[s])

    # phase A in 2x[128,1024] ps_s tiles; phase B in ps_ot/ps_acc slots
    # so it does not wait on phase A's PSUM->SBUF copies.
    pkA = [ps_s.tile([128, 1024], f32, tag="score", name=f"pkA{i}")
           for i in range(2)]
    kproj_pair(0, 1, [pkA[0][:, 0:512], pkA[0][:, 512:1024],
                      pkA[1][:, 0:512], pkA[1][:, 512:1024]], warmup=7)
    pkO = [ps_ot.tile([128, TLOC], f32, tag="ot", name=f"pkO{i}")
           for i in range(2)]
    pkB = [ps_acc.tile([128, 512], f32, tag="acc", name=f"pkB{i}")
           for i in range(2)]
    kproj_pair(2, 3, [pkO[0][:], pkO[1][:], pkB[0][:], pkB[1][:]])

    # ---------------- P1b: v projections ------------------------------
    # chunk 7 is deferred into head-0's iteration as PE filler (the
    # pipelined head loop has no PV/rT work for h=0 yet).
    wvt = wp.tile([128, DC, 512], f16, tag="wv", name="wvt", bufs=1)
    nc.sync.dma_start(wvt[:], wv_d.ap())
    for jc in range(NJC - 1):
        pv = ps_acc.tile([128, 512], f32, tag="acc")
        for dc in range(DC):
            mm(pv[:], xt[:, dc, jc * 128:(jc + 1) * 128], wvt[:, dc, :],
               start=(dc == 0), stop=(dc == DC - 1))
        nc.vector.tensor_copy(vv[:, jc, :], pv[:])

    ps_acc.release()

    if stop_after == "kv":
        if lp is not None:
            lp.__exit__(None, None, None)
            dtile = pers.tile([128, 128], f32, tag="dtile")
            nc.vector.memset(dtile[:], 0.0)
            nc.sync.dma_start(dummy_d.ap(), dtile[0:1, :])
        wp.release()
        xp.release()
        proj.release()
        ps_ot.release()
        ps_s.release()
        pers.release()
        return

    # ---------------- P2: attention, q projection interleaved ---------
    attn = tc.alloc_tile_pool(name="attn", bufs=1, side="right")
    # Wo stream pool allocated before sm/ps_l (right-side pools are
    # released in stack order) so the first two 2 MiB chunks can be
    # prefetched during the attention tail.
    wop = tc.alloc_tile_pool(name="wo_pool", bufs=2, side="right")
    sm = tc.alloc_tile_pool(name="sm", bufs=2, side="right")
    ps_l = tc.alloc_tile_pool(name="ps_l", bufs=1, space="PSUM",
                              side="right")

    oT = attn.tile([128, H, TLOC], f16, tag="oT")
    # wT double-buffered by head parity: exps of head h write wTs[h%2]
    # while PV of head h-1 still reads wTs[(h-1)%2].
    wTs = [attn.tile([128, NJC, TLOC], f16, tag=f"wT{i}", name=f"wT{i}")
           for i in range(2)]

    woc_tiles = {}

    def wo_load(dblk):
        woc = wop.tile([128, H, 512], f16, tag="wo", name=f"wo{dblk}")
        nc.sync.dma_start(
            woc[:], wo_d.ap()[:, dblk * H * 512:(dblk + 1) * H * 512])
        woc_tiles[dblk] = woc

    def wo_load_half(dblk, half):
        # half-loads issued across two iterations: a monolithic 2 MiB
        # wo transfer (6.3us) queued at an iteration boundary lands in
        # front of the NEXT iteration's wqh load on the serial DMA
        # pipe and stalls its qp chunks
        if half == 0:
            woc = wop.tile([128, H, 512], f16, tag="wo", name=f"wo{dblk}")
            woc_tiles[dblk] = woc
        woc = woc_tiles[dblk]
        lo = dblk * H * 512 + half * (H // 2) * 512
        nc.sync.dma_start(
            woc[:, half * (H // 2):(half + 1) * (H // 2), :],
            wo_d.ap()[:, lo:lo + (H // 2) * 512])

    # --- per-head emission pieces (closures over head state) ---------

    def make_scores_T(h):
        """Transposed-score emitters for head h. pair(pi) computes the
        sT blocks of chunks (pi, pi+4) in one [128,1024] PSUM tile
        (2 matmuls streaming qT columns), applies both edge masks with
        one strided DVE add, and exps into wT[:, jc, .] directly.
        lsum(t) accumulates the masked row sums via 5 N=1 matmuls.
        fin() adds lcorr and takes the reciprocal."""
        g = h // N_REP
        wT = wTs[h % 2]
        l_ps = ps_l.tile([128, NT], f32, tag="l", name=f"l{h}", bufs=1)

        def pair(pi):
            jc0, jc1 = pi, pi + 4
            n0, n1 = pi + 1, 4 - pi
            ps = ps_s.tile([128, 1024], f32, tag="score",
                           name=f"sT{h}_{pi}")
            # chunk jc0 covers q-tiles 0..pi at cols [0, n0*128);
            # chunk jc1 covers q-tiles pi..3 at cols [512, 512+n1*128)
            mm(ps[:, 0:n0 * 128], kT[:, g, jc0 * 128:(jc0 + 1) * 128],
               qT[:, h, 0:n0 * 128], start=True, stop=True)
            mm(ps[:, 512:512 + n1 * 128],
               kT[:, g, jc1 * 128:(jc1 + 1) * 128],
               qT[:, h, pi * 128:TLOC], start=True, stop=True)
            # exp the RAW band scores immediately (scores are O(1), no
            # overflow); the window mask is applied afterwards by
            # mask_fix on the otherwise-idle Pool engine, keeping the
            # matmul->exp->slot-free chain as short as possible.
            nc.scalar.activation(wT[:, jc0, 0:n0 * 128], ps[:, 0:n0 * 128],
                                 mybir.ActivationFunctionType.Exp,
                                 scale=SCALE)
            nc.scalar.activation(wT[:, jc1, pi * 128:TLOC],
                                 ps[:, 512:512 + n1 * 128],
                                 mybir.ActivationFunctionType.Exp,
                                 scale=SCALE)
            # zero the disallowed triangles of this pair's two edge
            # blocks right away on the (otherwise idle) Pool engine, so
            # next iteration's PV/lsum never wait on a late mask pass
            # (partition p = key jj, col = query ii)
            e0 = wT[:, jc0, pi * 128:(pi + 1) * 128]
            nc.gpsimd.affine_select(
                out=e0, in_=e0, compare_op=mybir.AluOpType.is_ge,
                fill=0.0, base=0, pattern=[[-1, 128]],
                channel_multiplier=1)
            e4 = wT[:, jc1, pi * 128:(pi + 1) * 128]
            nc.gpsimd.affine_select(
                out=e4, in_=e4, compare_op=mybir.AluOpType.is_ge,
                fill=0.0, base=0, pattern=[[1, 128]],
                channel_multiplier=-1)

        def lsum(t):
            # row sums l[i] for q-tile t: 5 one-column matmuls
            # (stationary = wT chunk, moving = ones) accumulating into
            # l_ps[:, t]. Stationary loads are pipelined; the charged
            # stream is 1 column per matmul.
            for i, jc in enumerate(range(t, t + 5)):
                mm(l_ps[:, t:t + 1], wT[:, jc, t * 128:(t + 1) * 128],
                   ones[:], start=(i == 0), stop=(i == 4))

        def fin():
            # emitted mid-iteration h+1 (after the interleaved
            # l-matmuls): by then those have retired, so these DVE ops
            # never block the in-order DVE queue.
            lf = sm.tile([128, NT], f32, tag="lf", name=f"lf{h}", bufs=2)
            r = sm.tile([128, NT], f32, tag="r", name=f"r{h}", bufs=2)
            nc.vector.tensor_add(lf[:], l_ps[:], lcorr_s[:])
            nc.vector.reciprocal(r[:], lf[:])
            return r

        return pair, lsum, fin, (h, wT)

    def make_qp(hq):
        """q-projection matmul chunks for head hq (PE filler)."""
        if hq >= H:
            return (lambda lo, hi: None), (lambda: None)
        wqh = wp.tile([128, DC, 128], f16, tag="wlhs", name=f"wqh{hq}",
                      bufs=4)
        nc.sync.dma_start(
            wqh[:], wq_d.ap()[:, hq * DC * 128:(hq + 1) * DC * 128])
        pq = ps_ot.tile([128, TLOC], f32, tag="ot", name=f"pq{hq}")

        def qp_mms(dc_lo, dc_hi):
            for dc in range(dc_lo, dc_hi):
                mm(pq[:], wqh[:, dc, :], xt[:, dc, WINDOW:XROWS],
                   start=(dc == 0), stop=(dc == DC - 1))

        def qp_fin():
            # (GPSIMD cannot access PSUM on HW)
            nc.scalar.copy(qT[:, hq, :], pq[:])

        return qp_mms, qp_fin

    def make_pv(state):
        """banded PV emitters for head h, reading wTs[h%2] (whose exps
        completed last iteration). PV is oriented [i, e] (stationary =
        wT chunk, moving = v chunk) so the 1/l normalization is a plain
        per-partition DVE tensor_scalar multiply with r -- no
        transpose/broadcast chain, and the po PSUM slot frees as soon
        as the multiplies run."""
        h, wT = state
        g = h // N_REP
        po = ps_ot.tile([128, TLOC], f32, tag="ot", name=f"po{h}")
        o_sb = sm.tile([128, NT, 128], f16, tag="osb", name=f"osb{h}",
                       bufs=2)

        def pv(t):
            # one accumulation group open at a time per PSUM bank
            for i, jc in enumerate(range(t, t + 5)):
                mm(po[:, t * 128:(t + 1) * 128],
                   wT[:, jc, t * 128:(t + 1) * 128],
                   vv[:, jc, g * 128:(g + 1) * 128],
                   start=(i == 0), stop=(i == 4))

        def onorm(t, r):
            nc.vector.tensor_scalar_mul(
                o_sb[:, t, :], po[:, t * 128:(t + 1) * 128], r[:, t:t + 1])

        def ot_fin():
            # o [i, e] -> oT [e, i] for P3: 4 PE transposes into one
            # PSUM bank + one DVE copy out. Deps are loose (o_sb is
            # stable SBUF), so this can ride anywhere in the next
            # iteration's PE stream.
            poT = ps_l.tile([128, TLOC], f16, tag="poT", name=f"poT{h}",
                            bufs=1)
            for t in range(NT):
                nc.tensor.transpose(poT[:, t * 128:(t + 1) * 128],
                                    o_sb[:, t, :], ident[:])
            nc.vector.tensor_copy(oT[:, h, :], poT[:])

        return pv, onorm, ot_fin

    py_tiles = {}  # t -> (tile, col half)

    def py_open(dblk):
        pyA = ps_s.tile([128, 1024], f32, tag="score", name=f"pyA{dblk}")
        pyB = ps_s.tile([128, 1024], f32, tag="score", name=f"pyB{dblk}")
        for t in range(NT):
            py_tiles[t] = (pyA if t < 2 else pyB, t % 2)

    def py_mms(t, h_lo, h_hi, dblk):
        woc = woc_tiles[dblk]
        py, half = py_tiles[t]
        for h2 in range(h_lo, h_hi):
            mm(py[:, half * 512:(half + 1) * 512],
               oT[:, h2, t * 128:(t + 1) * 128], woc[:, h2, :],
               start=(h2 == 0), stop=(h2 == H - 1))

    skip_p3 = stop_after == "attn"

    def PY(t, h_lo, h_hi):
        if not skip_p3:
            py_mms(t, h_lo, h_hi, 0)

    def vv_late(jc, eng):
        """deferred v-projection chunk (PE filler in iteration 0)"""
        pvv = ps_ot.tile([128, TLOC], f32, tag="ot", name=f"pvv{jc}")
        for dc in range(DC):
            mm(pvv[:], xt[:, dc, jc * 128:(jc + 1) * 128],
               wvt[:, dc, :], start=(dc == 0), stop=(dc == DC - 1))
        if eng == 0:
            nc.vector.tensor_copy(vv[:, jc, :], pvv[:])
        else:
            nc.scalar.copy(vv[:, jc, :], pvv[:])

    # --- pipelined head loop -----------------------------------------
    # Iteration h emits: scoresT+exp of head h, q-proj of h+1, PV and
    # normalized oT copy-out of h-1, r-chain of h-1 (transpose early so
    # the Pool broadcasts finish before ot_fin needs them).

    # head 0's q-projection runs standalone (heads h+1 ride iteration h)
    qp0_mms, qp0_fin = make_qp(0)
    qp0_mms(0, DC)
    qp0_fin()

    prev = None     # ((h, wT), lsum, fin, onorm, ot_fin) for head h-1
    potfin = None   # ot_fin of head h-2 (emitted in iteration h)
    for h in range(H):
        pair, lsum, fin, cur_state = make_scores_T(h)
        qp_mms, qp_fin = make_qp(h + 1)
        if prev is not None:
            pstate, plsum, pfin, _, _ = prev
            pv, onorm, ot_fin = make_pv(pstate)
        else:
            pv = onorm = ot_fin = None

        def PV(t):
            if pv is not None:
                pv(t)

        # pair first: its qT dep retires early now (the wo half-load
        # split unclogged the wqh/qp chain), and running it ahead of
        # PV(0) hides PV's po-slot DVE sem propagation (~150ns) under
        # pair(0)'s matmuls
        pair(0)
        PV(0)
        if potfin is not None:
            potfin()   # oT transposes/copy of h-2; deps fully retired
        pair(1)
        PV(1)
        if h == 0:
            vv_late(7, 0)
        # qp chunks 0..7 with the h-1 l-matmuls interleaved: the tiny
        # N=1 matmuls decode while the PE engine streams the 512-col qp
        # chunks, so their fixed per-instruction latency is hidden. The
        # 1.7us of qp also gives exp(pair0/1) time to free the score
        # slots before pair(2)/pair(3) need them.
        for dc4 in range(4):
            qp_mms(dc4, dc4 + 1)
            if prev is not None:
                plsum(dc4)
        qp_mms(4, 8)
        if prev is not None:
            pr = pfin()
        pair(2)
        PV(2)
        pair(3)
        PV(3)
        # all four normalizing reads AFTER the last po write: Tile
        # tracks deps at bank granularity, so an onorm read interleaved
        # between PV writes makes the next PV wait on the whole
        # recip->onorm DVE chain
        if prev is not None:
            onorm(0, pr)
            onorm(1, pr)
            onorm(2, pr)
            onorm(3, pr)
        if h == H - 1 and not skip_p3:
            # no q-projection filler for a 17th head: use Wo block 0's
            # first partial accumulations instead (pyA/pyB bind to the
            # score buffers freed by this head's own exps)
            py_open(0)
            PY(0, 0, 7)
        qp_mms(8, DC)
        qp_fin()
        if 11 <= h <= 14:
            wo_load_half((h - 11) // 2, (h - 11) % 2)
        potfin = ot_fin
        prev = (cur_state, lsum, fin, onorm, ot_fin)

    # --- drain: l/PV/normalize of head 15, with the first Wo block's
    # partial accumulations (heads 0..13) as PE filler; the head-15
    # l-matmuls ride between the big PY/PV blocks.
    pstate, plsum, pfin, _, _ = prev
    pv15, onorm15, ot_fin15 = make_pv(pstate)
    pv15(0)
    plsum(0)
    if potfin is not None:
        potfin()   # oT of head 14
    PY(0, 7, 14)
    plsum(1)
    pv15(1)
    plsum(2)
    PY(1, 0, 7)
    plsum(3)
    pr15 = pfin()
    pv15(2)
    PY(1, 7, 14)
    pv15(3)
    onorm15(0, pr15)
    onorm15(1, pr15)
    onorm15(2, pr15)
    onorm15(3, pr15)
    ot_fin15()
    PY(2, 0, 7)
    PY(2, 7, 14)
    PY(3, 0, 7)
    PY(3, 7, 14)

    sm.release()
    ps_l.release()
    wp.release()
    xp.release()
    proj.release()

    if stop_after == "attn":
        if lp is not None:
            lp.__exit__(None, None, None)
            dtile = pers.tile([128, 128], f32, tag="dtile")
            nc.vector.memset(dtile[:], 0.0)
            nc.sync.dma_start(dummy_d.ap(), dtile[0:1, :])
        wop.release()
        attn.release()
        ps_ot.release()
        ps_s.release()
        pers.release()
        return

    # ---------------- P3: output projection ----------------
    # dblk 0's heads 0..13 already accumulated during the drain above;
    # finish with heads 14/15, then stream the remaining Wo blocks.
    def y_out(t, dblk, py_ap, split=False):
        ych = attn.tile([128, 512], f16, tag="ych", bufs=4)
        if split:
            # the very last output chunk: halve the copy->DMA chain so
            # the kernel-tail serial latency (copy + HWDGE + transfer +
            # DMA-sem propagation) is paid on 256 cols, not 512
            for half in range(2):
                cols = slice(half * 256, (half + 1) * 256)
                nc.vector.tensor_copy(ych[:, cols], py_ap[:, cols])
                nc.scalar.dma_start(
                    y_d.ap()[t * 128:(t + 1) * 128,
                             dblk * 512 + half * 256:
                             dblk * 512 + (half + 1) * 256],
                    ych[:, cols])
            return
        nc.vector.tensor_copy(ych[:], py_ap)
        nc.scalar.dma_start(
            y_d.ap()[t * 128:(t + 1) * 128,
                     dblk * 512:(dblk + 1) * 512],
            ych[:])

    # all h=14 contributions first (oT14 ready early), so the PE has
    # work while the normalized oT15 multiply drains
    for t in range(NT):
        py_mms(t, H - 2, H - 1, 0)
    for t in range(NT):
        py_mms(t, H - 1, H, 0)
        py, half = py_tiles[t]
        y_out(t, 0, py[:, half * 512:(half + 1) * 512])

    for dblk in range(1, 4):
        if dblk not in woc_tiles:
            wo_load(dblk)
        woc = woc_tiles[dblk]
        for t in range(NT):
            # alternate PSUM pools between dblks so a block's first
            # matmuls never wait on the previous block's output copies
            if dblk % 2 == 0:
                py = ps_s.tile([128, 1024], f32, tag="score",
                               name=f"py{dblk}_{t}")
                py_ap = py[:, 0:512]
            else:
                py = ps_ot.tile([128, TLOC], f32, tag="ot",
                                name=f"py{dblk}_{t}")
                py_ap = py[:]
            for h in range(H):
                mm(py_ap, oT[:, h, t * 128:(t + 1) * 128],
                   woc[:, h, :], start=(h == 0), stop=(h == H - 1))
            y_out(t, dblk, py_ap)

    wop.release()
    attn.release()

    if lp is not None:
        lp.__exit__(None, None, None)
        dtile = pers.tile([128, 128], f32, tag="dtile")
        nc.vector.memset(dtile[:], 0.0)
        nc.sync.dma_start(dummy_d.ap(), dtile[0:1, :])

    ps_ot.release()
    ps_s.release()
    pers.release()


def build_nc(loop_n=None, stop_after=None):
    key = ("nc", loop_n, stop_after)
    if key in _CACHE:
        return _CACHE[key]
    import concourse.bacc as bacc
    import concourse.mybir as mybir
    import concourse.tile as tile
    from concourse.masks import make_identity

    nc = bacc.Bacc("TRN2", target_bir_lowering=False, debug=False,
                   num_devices=N_CORES)
    with tile.TileContext(nc) as tc:
        _emit(nc, tc, tile, mybir, make_identity, loop_n=loop_n,
              stop_after=stop_after)
    nc.compile()
    _CACHE[key] = nc
    return nc


def make_inputs_for_core(c, xf, Wq, Wk, Wv, Wo):
    """xf: [T, D] float32 (already squeezed)."""
    f16 = np.float16
    if c == 0:
        x_c = np.concatenate(
            [np.zeros((WINDOW, D), np.float32), xf[:TLOC]], axis=0)
    else:
        x_c = xf[TLOC * c - WINDOW: TLOC * c + TLOC]

    # xt[p, dc, j] = x_c[j, 128*dc+p]
    xt = np.ascontiguousarray(
        x_c.reshape(XROWS, DC, 128).transpose(2, 1, 0).astype(f16)
    ).reshape(128, DC * XROWS)
    # wq[p, h, dc, e] = Wq[128*dc+p, 128*h+e]
    wq = np.ascontiguousarray(
        Wq.reshape(DC, 128, H, 128).transpose(1, 2, 0, 3).astype(f16)
    ).reshape(128, H * DC * 128)
    # wk[p, g, dc, e] = Wk[128*dc+p, 128*g+e]
    wk = np.ascontiguousarray(
        Wk.reshape(DC, 128, KVH, 128).transpose(1, 2, 0, 3).astype(f16)
    ).reshape(128, KVH * DC * 128)
    # wv[p, dc, e] = Wv[128*dc+p, e]
    wv = np.ascontiguousarray(
        Wv.reshape(DC, 128, KVH * HD).transpose(1, 0, 2).astype(f16)
    ).reshape(128, DC * 512)
    # wo[p, dblk, h, e] = Wo[128*h+p, 512*dblk+e]
    wo = np.ascontiguousarray(
        Wo.reshape(H, 128, 4, 512).transpose(1, 2, 0, 3).astype(f16)
    ).reshape(128, 4 * H * 512)

    # core 0: rows see (512 - i) spurious zero-halo keys, each exp(0)=1
    lcorr = np.zeros((128, NT), np.float32)
    if c == 0:
        p = np.arange(128)[:, None]
        t = np.arange(NT)[None, :]
        lcorr = -np.maximum(0, (512 - 128 * t) - p).astype(np.float32)

    return {
        "xt": xt,
        "wq": wq,
        "wk": wk,
        "wv": wv,
        "wo": wo,
        "lcorr": np.ascontiguousarray(lcorr),
    }


def kernel(x, Wq, Wk, Wv, Wo):
    from concourse.bass_utils import run_bass_kernel_spmd

    nc = build_nc()
    xf = np.asarray(x, np.float32).reshape(T, D)
    Wq = np.asarray(Wq, np.float32)
    Wk = np.asarray(Wk, np.float32)
    Wv = np.asarray(Wv, np.float32)
    Wo = np.asarray(Wo, np.float32)
    in_maps = [make_inputs_for_core(c, xf, Wq, Wk, Wv, Wo)
               for c in range(N_CORES)]
    res = run_bass_kernel_spmd(nc, in_maps, core_ids=list(range(N_CORES)))
    y = np.concatenate(
        [res.results[c]["y"].astype(np.float32) for c in range(N_CORES)],
        axis=0)
    return y.reshape(1, T, D)
